# revision 1
# baseline (speedup 1.0000x reference)
"""Trainium2 Bass kernel for nn_DateParser (bidirectional-LSTM encoder +
attention decoder). Data-parallel over batch: 1024 batch -> 8 cores x 128.

Device (Bass/Tile, SPMD on 8 cores): the 512-step bidirectional LSTM
encoder, which dominates FLOPs. Transposed layout (gate dim on
partitions, batch on free). Sigmoid is computed on the Tanh table
(sigma(x) = 0.5 + 0.5*tanh(x/2)) with the 0.5 input scales and gate
biases folded into the weights host-side, so one activation-table set
serves the whole kernel and each gate tile needs a single plain-tanh op.

Host: attention decoder (TY=32 steps) in vectorized numpy, and the final
softmax over the batch axis (which spans all cores).
"""

import numpy as np

B, TX, TY = 1024, 512, 32
NA, NS = 64, 128
VIN, VOUT = 64, 32
NCORES = 8
BL = B // NCORES          # 128 batch per core
TC = 32                   # time-chunk for X streaming
NCHUNK = TX // TC

_CACHE = {}


def _build(nc_mod):
    """Build + compile the per-core encoder program once."""
    import concourse.bass as bass
    import concourse.bacc as bacc
    import concourse.mybir as mybir
    from concourse import tile

    nc = bacc.Bacc("TRN2", target_bir_lowering=False, debug=False,
                   num_devices=NCORES)
    dt = mybir.dt.float32

    xt = nc.dram_tensor("xt", [TX, VIN, BL], dt, kind="ExternalInput").ap()
    # weight tensors: per dir, A=(i,f) block and B=(g,o) block
    wx = {}
    wh = {}
    for d in ("f", "b"):
        wx[d] = nc.dram_tensor(f"wx{d}", [VIN + 1, 4 * NA], dt,
                               kind="ExternalInput").ap()
        wh[d] = nc.dram_tensor(f"wh{d}", [NA, 4 * NA], dt,
                               kind="ExternalInput").ap()
    pre = nc.dram_tensor("pre", [TX, 2 * NA, BL], dt, kind="ExternalOutput").ap()

    TH = mybir.ActivationFunctionType.Tanh

    with tile.TileContext(nc) as tc:
        with (
            tc.tile_pool(name="const", bufs=1) as cpool,
            tc.tile_pool(name="xbuf", bufs=1) as xpool,
            tc.tile_pool(name="work", bufs=4) as wkpool,
            tc.tile_pool(name="hout", bufs=8) as hpool,
            tc.tile_pool(name="psum", bufs=4, space="PSUM") as psum,
        ):
            # weights resident in SBUF
            wx_sb = {}
            wh_sb = {}
            for d in ("f", "b"):
                t1 = cpool.tile([VIN + 1, 4 * NA], dt, name=f"wx{d}", tag=f"wx{d}")
                nc.sync.dma_start(t1[:], wx[d][:])
                wx_sb[d] = t1
                t2 = cpool.tile([NA, 4 * NA], dt, name=f"wh{d}", tag=f"wh{d}")
                nc.sync.dma_start(t2[:], wh[d][:])
                wh_sb[d] = t2

            # x stream buffers (double-buffered, per dir), row VIN = ones
            xbuf = {}
            for d in ("f", "b"):
                for s in (0, 1):
                    t = xpool.tile([VIN + 1, TC, BL], dt, name=f"x{d}{s}", tag=f"x{d}{s}")
                    nc.gpsimd.memset(t[VIN:VIN + 1, :, :], 1.0)
                    xbuf[d, s] = t

            h0 = cpool.tile([NA, BL], dt, name="h0", tag="h0")
            nc.gpsimd.memset(h0[:], 0.0)
            cstate = {}
            for d in ("f", "b"):
                cstate[d] = cpool.tile([NA, BL], dt, name=f"c{d}", tag=f"c{d}")
                nc.gpsimd.memset(cstate[d][:], 0.0)

            hprev = {"f": h0, "b": h0}

            for c in range(NCHUNK):
                # fwd chunk c covers times [TC*c, TC*(c+1))
                nc.sync.dma_start(
                    xbuf["f", c % 2][0:VIN, :, :],
                    xt[TC * c:TC * (c + 1), :, :].rearrange("t v b -> v t b"),
                )
                # bwd chunk c covers times [TX - TC*(c+1), TX - TC*c)
                nc.sync.dma_start(
                    xbuf["b", c % 2][0:VIN, :, :],
                    xt[TX - TC * (c + 1):TX - TC * c, :, :].rearrange(
                        "t v b -> v t b"),
                )
                for tl in range(TC):
                    for d in ("f", "b"):
                        if d == "f":
                            t_actual = TC * c + tl
                            xcol = tl
                        else:
                            t_actual = TX - 1 - (TC * c + tl)
                            xcol = TC - 1 - tl
                        xrhs = xbuf[d, c % 2][:, xcol, :]
                        # four m=64 blocks so every gate sits on partitions
                        # 0-63 (walrus: DVE two-input ops need equal base
                        # partition); column blocks [i|f|g|o]
                        z = psum.tile([NA, 4 * BL], dt, name="z", tag="z")
                        for g in range(4):
                            cs = slice(g * BL, (g + 1) * BL)
                            ws = slice(g * NA, (g + 1) * NA)
                            nc.tensor.matmul(z[:, cs], wx_sb[d][:, ws], xrhs,
                                             start=True, stop=False)
                            nc.tensor.matmul(z[:, cs], wh_sb[d][:, ws],
                                             hprev[d][:], start=False,
                                             stop=True)
                        T = wkpool.tile([NA, 4 * BL], dt, name="T", tag="T")
                        nc.scalar.activation(T[:], z[:], TH)
                        ti = T[:, 0:BL]
                        tf = T[:, BL:2 * BL]
                        tg = T[:, 2 * BL:3 * BL]
                        to = T[:, 3 * BL:4 * BL]
                        # doubled state: cstate holds C' = 2c, h tiles hold
                        # H' = 2h (Wh pre-scaled 0.5 on host; pre *= 0.5 on
                        # host).  sigma(x) = (1 + tanh(x/2)) / 2.
                        m1 = wkpool.tile([NA, BL], dt, name="m1", tag="m1")
                        m2 = wkpool.tile([NA, BL], dt, name="m2", tag="m2")
                        AD, MU = mybir.AluOpType.add, mybir.AluOpType.mult
                        # m1 = (tf + 1) * C'   (= 4*sigmoid(f)*c)
                        nc.vector.scalar_tensor_tensor(m1[:], tf, 1.0,
                                                       cstate[d][:], AD, MU)
                        # m2 = (ti + 1) * tg   (= 2*sigmoid(i)*tanh(g))
                        nc.vector.scalar_tensor_tensor(m2[:], ti, 1.0, tg,
                                                       AD, MU)
                        # C'_new = 0.5*m1 + m2 = 2*c_new
                        nc.vector.scalar_tensor_tensor(cstate[d][:], m1[:],
                                                       0.5, m2[:], MU, AD)
                        tcell = wkpool.tile([NA, BL], dt, name="tc", tag="tc")
                        # tanh(c_new) = tanh(0.5 * C'_new)
                        nc.scalar.activation(tcell[:], cstate[d][:], TH,
                                             scale=0.5)
                        hnew = hpool.tile([NA, BL], dt, name="h", tag="h")
                        # H' = (to + 1) * tanh(c) = 2h
                        nc.vector.scalar_tensor_tensor(hnew[:], to, 1.0,
                                                       tcell[:], AD, MU)
                        f0 = 0 if d == "f" else NA
                        nc.sync.dma_start(pre[t_actual, f0:f0 + NA, :],
                                          hnew[:])
                        hprev[d] = hnew

    nc.compile()
    return nc


def _get_nc():
    if "nc" not in _CACHE:
        _CACHE["nc"] = _build(None)
    return _CACHE["nc"]


def _prep_weights(Wih, Whh, bih, bhh):
    """Fold the sigmoid-from-tanh 0.5 scales and the gate bias into the
    matmul weights.  Gate order i,f,g,o (64 each).  Returns per-block
    (wx_aug (65,128) with bias row, wh (64,128)) for A=(i,f), B=(g,o)."""
    b = (bih + bhh).astype(np.float32)
    scale = np.concatenate([np.full(2 * NA, 0.5, np.float32),
                            np.full(NA, 1.0, np.float32),
                            np.full(NA, 0.5, np.float32)])
    Wx = (Wih * scale[:, None]).astype(np.float32)       # (256, 64)
    Wh = (Whh * (0.5 * scale)[:, None]).astype(np.float32)  # (256,64); extra 0.5: rhs is H'=2h
    bb = (b * scale).astype(np.float32)                  # (256,)
    wx_aug = np.concatenate([Wx.T, bb[None, :]], axis=0)   # (65, 256)
    return (np.ascontiguousarray(wx_aug), np.ascontiguousarray(Wh.T))


import time as _time


def _run_cached(nc, in_maps):
    """run_bass_via_pjrt with the jitted sharded callable cached across
    calls (the library re-traces and re-jits every invocation)."""
    import jax
    import numpy as _np
    from jax.sharding import Mesh, PartitionSpec
    from jax.experimental.shard_map import shard_map
    from concourse import bass2jax as b2j

    if "runner" not in _CACHE:
        b2j.install_neuronx_cc_hook()
        import concourse.mybir as mybir
        pname = (nc.partition_id_tensor.name
                 if nc.partition_id_tensor else None)
        in_names, out_names, out_avals = [], [], []
        for alloc in nc.m.functions[0].allocations:
            if not isinstance(alloc, mybir.MemoryLocationSet):
                continue
            name = alloc.memorylocations[0].name
            if alloc.kind == "ExternalInput":
                if name != pname:
                    in_names.append(name)
            elif alloc.kind == "ExternalOutput":
                out_names.append(name)
                out_avals.append(jax.core.ShapedArray(
                    tuple(alloc.tensor_shape), mybir.dt.np(alloc.dtype)))
        n_params = len(in_names)
        all_names = in_names + out_names
        if pname is not None:
            all_names = all_names + [pname]

        def _body(*args):
            ops = list(args)
            if pname is not None:
                ops.append(b2j.partition_id_tensor())
            outs = b2j._bass_exec_p.bind(
                *ops, out_avals=tuple(out_avals), in_names=tuple(all_names),
                out_names=tuple(out_names), lowering_input_output_aliases=(),
                sim_require_finite=True, sim_require_nnan=True, nc=nc)
            return tuple(outs)

        devices = jax.devices()[:NCORES]
        mesh = Mesh(_np.asarray(devices), ("core",))
        nio = n_params + len(out_names)
        sharded = jax.jit(
            shard_map(_body, mesh=mesh,
                      in_specs=(PartitionSpec("core"),) * nio,
                      out_specs=(PartitionSpec("core"),) * len(out_names),
                      check_rep=False),
            donate_argnums=tuple(range(n_params, nio)), keep_unused=True)
        _CACHE["runner"] = (sharded, in_names, out_names, out_avals, n_params)

    sharded, in_names, out_names, out_avals, n_params = _CACHE["runner"]
    concat_in = [_np.concatenate([_np.asarray(m[n]) for m in in_maps], axis=0)
                 for n in in_names]
    concat_zeros = [
        _np.zeros((NCORES * a.shape[0], *a.shape[1:]), a.dtype)
        for a in out_avals]
    out_arrs = sharded(*concat_in, *concat_zeros)
    return [
        {n: _np.asarray(out_arrs[i]).reshape(NCORES, *out_avals[i].shape)[c]
         for i, n in enumerate(out_names)}
        for c in range(NCORES)
    ]


def kernel(X, Wih_f, Whh_f, bih_f, bhh_f, Wih_b, Whh_b, bih_b, bhh_b,
           Wih_p, Whh_p, bih_p, bhh_p, W1, b1, W2, b2, W3, b3):
    from concourse.bass_utils import run_bass_kernel_spmd

    _t = {}; _t0 = _time.time()
    nc = _get_nc()
    _t['build'] = _time.time() - _t0; _t0 = _time.time()

    wf = _prep_weights(Wih_f, Whh_f, bih_f, bhh_f)
    wb = _prep_weights(Wih_b, Whh_b, bih_b, bhh_b)

    in_maps = []
    for c in range(NCORES):
        xc = X[c * BL:(c + 1) * BL]                      # (128, 512, 64)
        xtc = np.ascontiguousarray(xc.transpose(1, 2, 0).astype(np.float32))
        m = {"xt": xtc}
        for d, w in (("f", wf), ("b", wb)):
            m[f"wx{d}"] = w[0]
            m[f"wh{d}"] = w[1]
        in_maps.append(m)

    _t['prep'] = _time.time() - _t0; _t0 = _time.time()
    try:
        results = _run_cached(nc, in_maps)
    except Exception:
        results = run_bass_kernel_spmd(
            nc, in_maps, core_ids=list(range(NCORES))).results
    _t['spmd'] = _time.time() - _t0; _t0 = _time.time()
    _CACHE["last_results"] = results
    _CACHE["last_in_maps"] = in_maps

    # assemble pre_out (B, TX, 2*NA)
    pre = np.empty((B, TX, 2 * NA), np.float32)
    for c in range(NCORES):
        p = results[c]["pre"]                        # (512, 128, 128)
        pre[c * BL:(c + 1) * BL] = p.transpose(2, 0, 1)

    # ---- host decoder (vectorized numpy) ----
    bp = (bih_p + bhh_p).astype(np.float32)
    W1a = W1[:, :NS].astype(np.float32)                  # (10, 128) state part
    W1b = 0.5 * W1[:, NS:].astype(np.float32)            # (10,128); 0.5: pre holds 2h
    _t['assemble'] = _time.time() - _t0; _t0 = _time.time()
    PP = (pre.reshape(B * TX, NS) @ W1b.T).reshape(B, TX, 10) + b1
    s = np.zeros((B, NS), np.float32)
    cc = np.zeros((B, NS), np.float32)
    WihT = 0.5 * Wih_p.T.astype(np.float32)  # 0.5: ctx from 2h-scaled pre
    WhhT = Whh_p.T.astype(np.float32)
    W3T = W3.T.astype(np.float32)
    outs = np.empty((TY, B, VOUT), np.float32)

    def sig(v):
        return 1.0 / (1.0 + np.exp(-v))

    for t in range(TY):
        PS = s @ W1a.T                                   # (B, 10)
        e = np.tanh(PP + PS[:, None, :])
        q = (e @ W2.T)[:, :, 0] + b2[0]                  # (B, TX)
        u = np.maximum(q, 0.0)
        a = np.exp(u)
        a /= a.sum(axis=1, keepdims=True)
        ctx = np.einsum("bt,btf->bf", a, pre, optimize=True)
        z = ctx @ WihT + s @ WhhT + bp
        zi, zf, zg, zo = np.split(z, 4, axis=-1)
        cc = sig(zf) * cc + sig(zi) * np.tanh(zg)
        s = sig(zo) * np.tanh(cc)
        L = s @ W3T + b3
        em = np.exp(L - L.max(axis=0, keepdims=True))
        outs[t] = em / em.sum(axis=0, keepdims=True)

    _t['decoder'] = _time.time() - _t0
    _CACHE['timers'] = _t
    return np.ascontiguousarray(outs.transpose(1, 0, 2))



# revision 2
# speedup vs baseline: 1.6625x; 1.6625x over previous
"""Trainium2 Bass kernel for nn_DateParser — fully fused on-device.

Data-parallel over batch: 1024 -> 8 cores x 128. The axon tunnel moves
~50 MB/s, so the whole model (bidirectional LSTM encoder + attention
decoder) runs on-device; only X (bf16, 67 MB) goes up and logits (4 MB)
come back. Host does just the final softmax over the batch axis (it
spans all cores).

Encoder: transposed layout (gate dim on partitions, batch on free),
sigmoid via tanh identity with scales/biases folded into weights
(one activation table set: exp_and_others covers tanh/exp/relu/copy).
h is stored doubled (H' = 2h) so sigma(x) = (1+tanh(x/2))/2 needs no
extra multiply; all downstream weights absorb the 0.5.

Decoder (on device, batch-on-partitions layout): per step
  PS = s'@W1a' + b1                  (PE; W1a' pre-halved)
  e = tanh(PP + PS)                  (DVE add w/ bcast + ACT tanh)
  q = sum_k e*W2                     (DVE mult w/ bcast + reduce)
  w = exp(relu(q + b2)), Z = sum(w)  (ACT with accum)
  ctx = (sum_tx w * preB) / Z        (DVE bf16 mult+reduce, ACT scale)
  LSTM cell on [b, 4NS] gates        (PE matmuls + ACT/DVE pointwise)
  logits -> SBUF outbuf              (PE + DVE)
preB ([b, f, tx] bf16, 16.8MB) and PP ([b, tx, 10]) are built during
the encoder via PE transposes of the h tiles, so pre never leaves the
device.
"""

import numpy as np
import time as _time

B, TX, TY = 1024, 512, 32
NA, NS = 64, 128
VIN, VOUT = 64, 32
K1 = 10
NCORES = 8
BL = B // NCORES          # 128 batch per core
TC = 16                   # time-chunk for X streaming
NCHUNK = TX // TC
FC = 4                    # feature-chunk for ctx reduce

_CACHE = {}


def _build(unused=None):
    import concourse.bass as bass
    import concourse.bacc as bacc
    import concourse.mybir as mybir
    from concourse import tile
    from concourse.masks import make_identity

    nc = bacc.Bacc("TRN2", target_bir_lowering=False, debug=False,
                   num_devices=NCORES)
    f32 = mybir.dt.float32
    bf16 = mybir.dt.bfloat16
    TH = mybir.ActivationFunctionType.Tanh
    EX = mybir.ActivationFunctionType.Exp
    RL = mybir.ActivationFunctionType.Relu
    CP = mybir.ActivationFunctionType.Copy
    AD, MU = mybir.AluOpType.add, mybir.AluOpType.mult
    AX = mybir.AxisListType.X

    xt = nc.dram_tensor("xt", [TX, VIN, BL], bf16, kind="ExternalInput").ap()
    wx = {}
    wh = {}
    for d in ("f", "b"):
        wx[d] = nc.dram_tensor(f"wx{d}", [VIN + 1, 4 * NA], bf16,
                               kind="ExternalInput").ap()
        wh[d] = nc.dram_tensor(f"wh{d}", [NA, 4 * NA], bf16,
                               kind="ExternalInput").ap()
    w1pp_d = nc.dram_tensor("w1pp", [NA, 2 * K1], bf16, kind="ExternalInput").ap()
    w1a_d = nc.dram_tensor("w1a", [NS, K1], bf16, kind="ExternalInput").ap()
    b1r_d = nc.dram_tensor("b1r", [BL, K1], f32, kind="ExternalInput").ap()
    w2r_d = nc.dram_tensor("w2r", [BL, K1], bf16, kind="ExternalInput").ap()
    wihp_d = nc.dram_tensor("wihp", [NS, 4 * NS], bf16, kind="ExternalInput").ap()
    whhp_d = nc.dram_tensor("whhp", [NS, 4 * NS], bf16, kind="ExternalInput").ap()
    bpr_d = nc.dram_tensor("bpr", [BL, 4 * NS], f32, kind="ExternalInput").ap()
    w3t_d = nc.dram_tensor("w3t", [NS, VOUT], bf16, kind="ExternalInput").ap()
    b3r_d = nc.dram_tensor("b3r", [BL, VOUT], f32, kind="ExternalInput").ap()
    b2s_d = nc.dram_tensor("b2s", [BL, 1], f32, kind="ExternalInput").ap()

    logit = nc.dram_tensor("logit", [BL, TY * VOUT], f32,
                           kind="ExternalOutput").ap()

    with tile.TileContext(nc) as tc:
        with (
            tc.tile_pool(name="const", bufs=1) as cpool,
            tc.tile_pool(name="big", bufs=1) as bigpool,
        ):
            # ---- persistent SBUF tensors ----
            preB = bigpool.tile([BL, NS, TX], bf16, name="preB", tag="preB")
            PP = bigpool.tile([BL, TX, K1], bf16, name="PP", tag="PP")
            nc.gpsimd.memset(PP[:], 0.0)
            outbuf = bigpool.tile([BL, TY * VOUT], f32, name="outbuf",
                                  tag="outbuf")
            ident = cpool.tile([128, 128], bf16, name="ident", tag="ident")
            make_identity(nc, ident)

            # ---- load weights ----
            def ld(pool, dram, shape, dt, tag):
                t = pool.tile(shape, dt, name=tag, tag=tag)
                nc.sync.dma_start(t[:], dram[:])
                return t

            wx_sb = {d: ld(cpool, wx[d], [VIN + 1, 4 * NA], bf16, f"wx{d}")
                     for d in ("f", "b")}
            wh_sb = {d: ld(cpool, wh[d], [NA, 4 * NA], bf16, f"wh{d}")
                     for d in ("f", "b")}
            w1pp = ld(cpool, w1pp_d, [NA, 2 * K1], bf16, "w1pp")
            w1a = ld(cpool, w1a_d, [NS, K1], bf16, "w1a")
            b1r = ld(cpool, b1r_d, [BL, K1], f32, "b1r")
            w2r = ld(cpool, w2r_d, [BL, K1], bf16, "w2r")
            wihp = ld(cpool, wihp_d, [NS, 4 * NS], bf16, "wihp")
            whhp = ld(cpool, whhp_d, [NS, 4 * NS], bf16, "whhp")
            bpr = ld(cpool, bpr_d, [BL, 4 * NS], f32, "bpr")
            w3t = ld(cpool, w3t_d, [NS, VOUT], bf16, "w3t")
            b3r = ld(cpool, b3r_d, [BL, VOUT], f32, "b3r")
            b2s = ld(cpool, b2s_d, [BL, 1], f32, "b2s")

            # ---------------- encoder ----------------
            with (
                tc.tile_pool(name="xbuf", bufs=1) as xpool,
                tc.tile_pool(name="work", bufs=2) as wkpool,
                tc.tile_pool(name="hout", bufs=8) as hpool,
                tc.tile_pool(name="zps", bufs=3, space="PSUM") as zpsum,
                tc.tile_pool(name="pps", bufs=2, space="PSUM") as ppsum,
                tc.tile_pool(name="tps", bufs=2, space="PSUM") as tpsum,
            ):
                xbuf = {}
                for d in ("f", "b"):
                    for s in (0, 1):
                        t = xpool.tile([VIN + 1, TC, BL], bf16,
                                       name=f"x{d}{s}", tag=f"x{d}{s}")
                        nc.gpsimd.memset(t[VIN:VIN + 1, :, :], 1.0)
                        xbuf[d, s] = t

                h0 = cpool.tile([NA, BL], bf16, name="h0", tag="h0")
                nc.gpsimd.memset(h0[:], 0.0)
                cstate = {}
                for d in ("f", "b"):
                    cstate[d] = cpool.tile([NA, BL], f32, name=f"c{d}",
                                           tag=f"c{d}")
                    nc.gpsimd.memset(cstate[d][:], 0.0)
                hprev = {"f": h0, "b": h0}

                for c in range(NCHUNK):
                    nc.sync.dma_start(
                        xbuf["f", c % 2][0:VIN, :, :],
                        xt[TC * c:TC * (c + 1), :, :].rearrange(
                            "t v b -> v t b"))
                    nc.sync.dma_start(
                        xbuf["b", c % 2][0:VIN, :, :],
                        xt[TX - TC * (c + 1):TX - TC * c, :, :].rearrange(
                            "t v b -> v t b"))
                    for tl in range(TC):
                        for di, d in enumerate(("f", "b")):
                            if d == "f":
                                t_act = TC * c + tl
                                xcol = tl
                            else:
                                t_act = TX - 1 - (TC * c + tl)
                                xcol = TC - 1 - tl
                            xrhs = xbuf[d, c % 2][:, xcol, :]
                            z = zpsum.tile([NA, 4 * BL], f32, name="z", tag="z")
                            for g in range(4):
                                cs = slice(g * BL, (g + 1) * BL)
                                ws = slice(g * NA, (g + 1) * NA)
                                nc.tensor.matmul(z[:, cs], wx_sb[d][:, ws],
                                                 xrhs, start=True, stop=False)
                                nc.tensor.matmul(z[:, cs], wh_sb[d][:, ws],
                                                 hprev[d][:], start=False,
                                                 stop=True)
                            T = wkpool.tile([NA, 4 * BL], f32, name="T", tag="T")
                            nc.scalar.activation(T[:], z[:], TH)
                            ti = T[:, 0:BL]
                            tf = T[:, BL:2 * BL]
                            tg = T[:, 2 * BL:3 * BL]
                            to = T[:, 3 * BL:4 * BL]
                            m1 = wkpool.tile([NA, BL], f32, name="m1", tag="m1")
                            m2 = wkpool.tile([NA, BL], f32, name="m2", tag="m2")
                            # C' = 2c; m1=(tf+1)*C'; m2=(ti+1)*tg;
                            # C'new = 0.5*m1 + m2
                            nc.vector.scalar_tensor_tensor(
                                m1[:], tf, 1.0, cstate[d][:], AD, MU)
                            nc.vector.scalar_tensor_tensor(
                                m2[:], ti, 1.0, tg, AD, MU)
                            nc.vector.scalar_tensor_tensor(
                                cstate[d][:], m1[:], 0.5, m2[:], MU, AD)
                            tcell = wkpool.tile([NA, BL], f32, name="tc",
                                                tag="tc")
                            nc.scalar.activation(tcell[:], cstate[d][:], TH,
                                                 scale=0.5)
                            hnew = hpool.tile([NA, BL], bf16, name="h", tag="h")
                            # H' = (to+1)*tanh(c) = 2h
                            nc.vector.scalar_tensor_tensor(
                                hnew[:], to, 1.0, tcell[:], AD, MU)
                            hprev[d] = hnew
                            # PP[:, t, :] += hT @ w1pp_d  (PE, N=10)
                            pp_ps = ppsum.tile([BL, K1], f32, name="pp",
                                               tag="pp")
                            nc.tensor.matmul(
                                pp_ps[:], hnew[:],
                                w1pp[:, di * K1:(di + 1) * K1],
                                start=True, stop=True)
                            nc.vector.tensor_tensor(
                                PP[:, t_act, :], pp_ps[:], PP[:, t_act, :], AD)
                            # preB[:, d*64:(d+1)*64, t] = hnew.T
                            hT = tpsum.tile([BL, NA], bf16, name="hT", tag="hT")
                            nc.tensor.transpose(hT[:], hnew[:],
                                                ident[0:NA, 0:NA])
                            nc.scalar.copy(
                                preB[:, di * NA:(di + 1) * NA, t_act], hT[:])

            # ---------------- decoder ----------------
            with (
                tc.tile_pool(name="dwork", bufs=1) as dpool,
                tc.tile_pool(name="ebuf", bufs=1) as epool,
                tc.tile_pool(name="prod", bufs=2) as prpool,
                tc.tile_pool(name="dz", bufs=2, space="PSUM") as dzps,
                tc.tile_pool(name="dsm", bufs=1, space="PSUM") as dsps,
                tc.tile_pool(name="dtr", bufs=1, space="PSUM") as dtps,
            ):
                s_fT = cpool.tile([NS, BL], bf16, name="sfT", tag="sfT")
                nc.gpsimd.memset(s_fT[:], 0.0)
                cdec = cpool.tile([BL, NS], f32, name="cdec", tag="cdec")
                nc.gpsimd.memset(cdec[:], 0.0)
                e = epool.tile([BL, TX, K1], bf16, name="e", tag="e")

                for t in range(TY):
                    # PS = s'@w1a + b1
                    ps_ps = dsps.tile([BL, K1], f32, name="ps", tag="ps")
                    nc.tensor.matmul(ps_ps[:], s_fT[:], w1a[:],
                                     start=True, stop=True)
                    PS = dpool.tile([BL, K1], f32, name="PS", tag="PS")
                    nc.vector.tensor_tensor(PS[:], ps_ps[:], b1r[:], AD)
                    # e = tanh(PP + PS)
                    nc.vector.tensor_tensor(
                        e[:], PP[:],
                        PS[:, None, :].broadcast_to([BL, TX, K1]), AD)
                    nc.scalar.activation(e[:], e[:], TH)
                    # q = sum_k e*W2 ; u = relu(q + b2); w = exp(u), Z
                    nc.vector.tensor_tensor(
                        e[:], e[:],
                        w2r[:, None, :].broadcast_to([BL, TX, K1]), MU)
                    q = dpool.tile([BL, TX], f32, name="q", tag="q")
                    nc.vector.tensor_reduce(q[:], e[:], AX, AD)
                    u = dpool.tile([BL, TX], f32, name="u", tag="u")
                    nc.scalar.activation(u[:], q[:], RL, bias=b2s)
                    wat = dpool.tile([BL, TX], bf16, name="wat", tag="wat")
                    Z = dpool.tile([BL, 1], f32, name="Z", tag="Z")
                    nc.scalar.activation(wat[:], u[:], EX, accum_out=Z[:])
                    # ctx_u[f] = sum_tx w*preB
                    ctx_u = dpool.tile([BL, NS], f32, name="ctxu", tag="ctxu")
                    for fc in range(NS // FC):
                        fs = slice(fc * FC, (fc + 1) * FC)
                        prod = prpool.tile([BL, FC, TX], bf16, name="prod",
                                           tag="prod")
                        nc.vector.tensor_tensor(
                            prod[:], preB[:, fs, :],
                            wat[:, None, :].broadcast_to([BL, FC, TX]), MU)
                        nc.vector.tensor_reduce(ctx_u[:, fs], prod[:], AX, AD)
                    # ctx = ctx_u / Z  (ACT copy w/ per-partition scale)
                    Zr = dpool.tile([BL, 1], f32, name="Zr", tag="Zr")
                    nc.vector.reciprocal(Zr[:], Z[:])
                    ctx = dpool.tile([BL, NS], bf16, name="ctx", tag="ctx")
                    nc.scalar.activation(ctx[:], ctx_u[:], CP, scale=Zr)
                    # ctx_fT
                    ctT_ps = dtps.tile([NS, BL], bf16, name="ctT", tag="ctT")
                    nc.tensor.transpose(ctT_ps[:], ctx[:], ident[:])
                    ctx_fT = dpool.tile([NS, BL], bf16, name="cfT", tag="cfT")
                    nc.vector.tensor_copy(ctx_fT[:], ctT_ps[:])
                    # z = ctx@wihp + s@whhp + bp
                    z_ps = dzps.tile([BL, 4 * NS], f32, name="zd", tag="zd")
                    nc.tensor.matmul(z_ps[:], ctx_fT[:], wihp[:],
                                     start=True, stop=False)
                    nc.tensor.matmul(z_ps[:], s_fT[:], whhp[:],
                                     start=False, stop=True)
                    zb = dpool.tile([BL, 4 * NS], f32, name="zb", tag="zb")
                    nc.vector.tensor_tensor(zb[:], z_ps[:], bpr[:], AD)
                    # gates: i,f scale 0.5 tanh; g scale 1; o scale 0.5
                    Tg8 = dpool.tile([BL, 4 * NS], f32, name="Tg8", tag="Tg8")
                    nc.scalar.activation(Tg8[:, 0:2 * NS], zb[:, 0:2 * NS],
                                         TH, scale=0.5)
                    nc.scalar.activation(Tg8[:, 2 * NS:3 * NS],
                                         zb[:, 2 * NS:3 * NS], TH)
                    nc.scalar.activation(Tg8[:, 3 * NS:4 * NS],
                                         zb[:, 3 * NS:4 * NS], TH, scale=0.5)
                    ti = Tg8[:, 0:NS]
                    tf = Tg8[:, NS:2 * NS]
                    tg = Tg8[:, 2 * NS:3 * NS]
                    to = Tg8[:, 3 * NS:4 * NS]
                    m1 = dpool.tile([BL, NS], f32, name="dm1", tag="dm1")
                    m2 = dpool.tile([BL, NS], f32, name="dm2", tag="dm2")
                    nc.vector.scalar_tensor_tensor(m1[:], tf, 1.0, cdec[:],
                                                   AD, MU)
                    nc.vector.scalar_tensor_tensor(m2[:], ti, 1.0, tg, AD, MU)
                    nc.vector.scalar_tensor_tensor(cdec[:], m1[:], 0.5, m2[:],
                                                   MU, AD)
                    tcl = dpool.tile([BL, NS], f32, name="dtc", tag="dtc")
                    nc.scalar.activation(tcl[:], cdec[:], TH, scale=0.5)
                    s_b = dpool.tile([BL, NS], bf16, name="sb", tag="sb")
                    nc.vector.scalar_tensor_tensor(s_b[:], to, 1.0, tcl[:],
                                                   AD, MU)
                    # s_fT = s_b.T
                    sT_ps = dtps.tile([NS, BL], bf16, name="sT", tag="sT")
                    nc.tensor.transpose(sT_ps[:], s_b[:], ident[:])
                    nc.vector.tensor_copy(s_fT[:], sT_ps[:])
                    # logits
                    L_ps = dsps.tile([BL, VOUT], f32, name="L", tag="L")
                    nc.tensor.matmul(L_ps[:], s_fT[:], w3t[:],
                                     start=True, stop=True)
                    nc.vector.tensor_tensor(
                        outbuf[:, t * VOUT:(t + 1) * VOUT], L_ps[:], b3r[:],
                        AD)

                nc.sync.dma_start(logit[:], outbuf[:])

    nc.compile()
    return nc


def _get_nc():
    if "nc" not in _CACHE:
        _CACHE["nc"] = _build()
    return _CACHE["nc"]


def _prep_enc_weights(Wih, Whh, bih, bhh, bf16):
    """Baseline folding: sigmoid-from-tanh 0.5 scales + bias row; Whh gets
    an extra 0.5 because the h it multiplies is stored doubled."""
    b = (bih + bhh).astype(np.float32)
    scale = np.concatenate([np.full(2 * NA, 0.5, np.float32),
                            np.full(NA, 1.0, np.float32),
                            np.full(NA, 0.5, np.float32)])
    Wx = (Wih * scale[:, None]).astype(np.float32)
    Wh = (Whh * (0.5 * scale)[:, None]).astype(np.float32)
    bb = (b * scale).astype(np.float32)
    wx_aug = np.concatenate([Wx.T, bb[None, :]], axis=0)
    return (np.ascontiguousarray(wx_aug).astype(bf16),
            np.ascontiguousarray(Wh.T).astype(bf16))


def _run_cached(nc, in_maps):
    import jax
    import numpy as _np
    from jax.sharding import Mesh, PartitionSpec
    from jax.experimental.shard_map import shard_map
    from concourse import bass2jax as b2j

    if "runner" not in _CACHE:
        b2j.install_neuronx_cc_hook()
        import concourse.mybir as mybir
        pname = (nc.partition_id_tensor.name
                 if nc.partition_id_tensor else None)
        in_names, out_names, out_avals = [], [], []
        for alloc in nc.m.functions[0].allocations:
            if not isinstance(alloc, mybir.MemoryLocationSet):
                continue
            name = alloc.memorylocations[0].name
            if alloc.kind == "ExternalInput":
                if name != pname:
                    in_names.append(name)
            elif alloc.kind == "ExternalOutput":
                out_names.append(name)
                out_avals.append(jax.core.ShapedArray(
                    tuple(alloc.tensor_shape), mybir.dt.np(alloc.dtype)))
        n_params = len(in_names)
        all_names = in_names + out_names
        if pname is not None:
            all_names = all_names + [pname]

        def _body(*args):
            ops = list(args)
            if pname is not None:
                ops.append(b2j.partition_id_tensor())
            outs = b2j._bass_exec_p.bind(
                *ops, out_avals=tuple(out_avals), in_names=tuple(all_names),
                out_names=tuple(out_names), lowering_input_output_aliases=(),
                sim_require_finite=True, sim_require_nnan=True, nc=nc)
            return tuple(outs)

        devices = jax.devices()[:NCORES]
        mesh = Mesh(_np.asarray(devices), ("core",))
        nio = n_params + len(out_names)
        sharded = jax.jit(
            shard_map(_body, mesh=mesh,
                      in_specs=(PartitionSpec("core"),) * nio,
                      out_specs=(PartitionSpec("core"),) * len(out_names),
                      check_rep=False),
            donate_argnums=tuple(range(n_params, nio)), keep_unused=True)
        _CACHE["runner"] = (sharded, in_names, out_names, out_avals, n_params)

    sharded, in_names, out_names, out_avals, n_params = _CACHE["runner"]
    concat_in = [_np.concatenate([_np.asarray(m[n]) for m in in_maps], axis=0)
                 for n in in_names]
    concat_zeros = [
        _np.zeros((NCORES * a.shape[0], *a.shape[1:]), a.dtype)
        for a in out_avals]
    out_arrs = sharded(*concat_in, *concat_zeros)
    return [
        {n: _np.asarray(out_arrs[i]).reshape(NCORES, *out_avals[i].shape)[c]
         for i, n in enumerate(out_names)}
        for c in range(NCORES)
    ]


def kernel(X, Wih_f, Whh_f, bih_f, bhh_f, Wih_b, Whh_b, bih_b, bhh_b,
           Wih_p, Whh_p, bih_p, bhh_p, W1, b1, W2, b2, W3, b3):
    import ml_dtypes
    bf16 = ml_dtypes.bfloat16

    _t = {}
    _t0 = _time.time()
    nc = _get_nc()
    _t['build'] = _time.time() - _t0
    _t0 = _time.time()

    wf = _prep_enc_weights(Wih_f, Whh_f, bih_f, bhh_f, bf16)
    wb = _prep_enc_weights(Wih_b, Whh_b, bih_b, bhh_b, bf16)

    W1 = np.asarray(W1, np.float32)
    # decoder weights (shared across cores). All "0.5" folds: the doubled
    # h/s/pre states.
    w1pp = np.concatenate([0.5 * W1[:, NS:NS + NA].T,
                           0.5 * W1[:, NS + NA:2 * NS].T],
                          axis=1).astype(bf16)                   # (64, 20)
    w1a = np.ascontiguousarray(0.5 * W1[:, :NS].T).astype(bf16)  # (128, 10)
    b1r = np.broadcast_to(np.asarray(b1, np.float32), (BL, K1)).copy()
    w2r = np.broadcast_to(np.asarray(W2[0], np.float32).astype(bf16),
                          (BL, K1)).copy()
    wihp = np.ascontiguousarray(0.5 * np.asarray(Wih_p, np.float32).T
                                ).astype(bf16)                   # (128, 512)
    whhp = np.ascontiguousarray(0.5 * np.asarray(Whh_p, np.float32).T
                                ).astype(bf16)
    bpr = np.broadcast_to((np.asarray(bih_p, np.float32)
                           + np.asarray(bhh_p, np.float32)), (BL, 4 * NS)
                          ).copy()
    w3t = np.ascontiguousarray(0.5 * np.asarray(W3, np.float32).T
                               ).astype(bf16)                    # (128, 32)
    b3r = np.broadcast_to(np.asarray(b3, np.float32), (BL, VOUT)).copy()
    b2s = np.full((BL, 1), float(np.asarray(b2).ravel()[0]), np.float32)

    Xb = np.asarray(X)
    in_maps = []
    for c in range(NCORES):
        xc = Xb[c * BL:(c + 1) * BL]                     # (128, 512, 64)
        xtc = np.ascontiguousarray(
            xc.transpose(1, 2, 0)).astype(bf16)          # (512, 64, 128)
        m = {"xt": xtc, "w1pp": w1pp, "w1a": w1a, "b1r": b1r, "w2r": w2r,
             "wihp": wihp, "whhp": whhp, "bpr": bpr, "w3t": w3t,
             "b3r": b3r, "b2s": b2s}
        for d, w in (("f", wf), ("b", wb)):
            m[f"wx{d}"] = w[0]
            m[f"wh{d}"] = w[1]
        in_maps.append(m)

    _t['prep'] = _time.time() - _t0
    _t0 = _time.time()
    try:
        results = _run_cached(nc, in_maps)
    except Exception:
        from concourse.bass_utils import run_bass_kernel_spmd
        results = run_bass_kernel_spmd(
            nc, in_maps, core_ids=list(range(NCORES))).results
    _t['spmd'] = _time.time() - _t0
    _t0 = _time.time()
    _CACHE["last_results"] = results
    _CACHE["last_in_maps"] = in_maps

    # logits (B, TY, VOUT); final softmax over batch axis
    L = np.concatenate(
        [results[c]["logit"].reshape(BL, TY, VOUT) for c in range(NCORES)],
        axis=0)
    L -= L.max(axis=0, keepdims=True)
    np.exp(L, out=L)
    L /= L.sum(axis=0, keepdims=True)
    _t['post'] = _time.time() - _t0
    _CACHE['timers'] = _t
    return np.ascontiguousarray(L)


# revision 3
# speedup vs baseline: 1.8949x; 1.1398x over previous
"""Trainium2 Bass kernel for nn_DateParser — fully fused on-device.

Data-parallel over batch: 1024 -> 8 cores x 128. The axon tunnel moves
~50 MB/s, so the whole model (bidirectional LSTM encoder + attention
decoder) runs on-device; only X (bf16, 67 MB) goes up and logits (4 MB)
come back. Host does just the final softmax over the batch axis (it
spans all cores).

Encoder: transposed layout (gate dim on partitions, batch on free),
sigmoid via tanh identity with scales/biases folded into weights
(one activation table set: exp_and_others covers tanh/exp/relu/copy).
h is stored doubled (H' = 2h) so sigma(x) = (1+tanh(x/2))/2 needs no
extra multiply; all downstream weights absorb the 0.5.

Decoder (on device, batch-on-partitions layout): per step
  PS = s'@W1a' + b1                  (PE; W1a' pre-halved)
  e = tanh(PP + PS)                  (DVE add w/ bcast + ACT tanh)
  q = sum_k e*W2                     (DVE mult w/ bcast + reduce)
  w = exp(relu(q + b2)), Z = sum(w)  (ACT with accum)
  ctx = (sum_tx w * preB) / Z        (DVE bf16 mult+reduce, ACT scale)
  LSTM cell on [b, 4NS] gates        (PE matmuls + ACT/DVE pointwise)
  logits -> SBUF outbuf              (PE + DVE)
preB ([b, f, tx] bf16, 16.8MB) and PP ([b, tx, 10]) are built during
the encoder via PE transposes of the h tiles, so pre never leaves the
device.
"""

import numpy as np
import time as _time

B, TX, TY = 1024, 512, 32
NA, NS = 64, 128
VIN, VOUT = 64, 32
K1 = 10
NCORES = 8
BL = B // NCORES          # 128 batch per core
TC = 16                   # time-chunk for X streaming
NCHUNK = TX // TC
FC = 4                    # feature-chunk for ctx reduce

_CACHE = {}


def _build(unused=None):
    import concourse.bass as bass
    import concourse.bacc as bacc
    import concourse.mybir as mybir
    from concourse import tile
    from concourse.masks import make_identity

    nc = bacc.Bacc("TRN2", target_bir_lowering=False, debug=False,
                   num_devices=NCORES)
    f32 = mybir.dt.float32
    bf16 = mybir.dt.bfloat16
    TH = mybir.ActivationFunctionType.Tanh
    EX = mybir.ActivationFunctionType.Exp
    RL = mybir.ActivationFunctionType.Relu
    CP = mybir.ActivationFunctionType.Copy
    AD, MU = mybir.AluOpType.add, mybir.AluOpType.mult
    AX = mybir.AxisListType.X

    fp8 = mybir.dt.float8e3
    xt = nc.dram_tensor("xt", [TX, VIN, BL], fp8, kind="ExternalInput").ap()
    wx = {}
    wh = {}
    for d in ("f", "b"):
        wx[d] = nc.dram_tensor(f"wx{d}", [VIN + 1, 4 * NA], bf16,
                               kind="ExternalInput").ap()
        wh[d] = nc.dram_tensor(f"wh{d}", [NA, 4 * NA], bf16,
                               kind="ExternalInput").ap()
    w1pp_d = nc.dram_tensor("w1pp", [NA, 2 * K1], bf16, kind="ExternalInput").ap()
    w1a_d = nc.dram_tensor("w1a", [NS, K1], bf16, kind="ExternalInput").ap()
    b1r_d = nc.dram_tensor("b1r", [BL, K1], f32, kind="ExternalInput").ap()
    w2r_d = nc.dram_tensor("w2r", [BL, K1], bf16, kind="ExternalInput").ap()
    wihp_d = nc.dram_tensor("wihp", [NS, 4 * NS], bf16, kind="ExternalInput").ap()
    whhp_d = nc.dram_tensor("whhp", [NS, 4 * NS], bf16, kind="ExternalInput").ap()
    bpr_d = nc.dram_tensor("bpr", [BL, 4 * NS], f32, kind="ExternalInput").ap()
    w3t_d = nc.dram_tensor("w3t", [NS, VOUT], bf16, kind="ExternalInput").ap()
    b3r_d = nc.dram_tensor("b3r", [BL, VOUT], f32, kind="ExternalInput").ap()
    b2s_d = nc.dram_tensor("b2s", [BL, 1], f32, kind="ExternalInput").ap()

    logit = nc.dram_tensor("logit", [BL, TY * VOUT], bf16,
                           kind="ExternalOutput").ap()

    with tile.TileContext(nc) as tc:
        with (
            tc.tile_pool(name="const", bufs=1) as cpool,
            tc.tile_pool(name="big", bufs=1) as bigpool,
        ):
            # ---- persistent SBUF tensors ----
            preB = bigpool.tile([BL, NS, TX], bf16, name="preB", tag="preB")
            PP = bigpool.tile([BL, TX, K1], bf16, name="PP", tag="PP")
            nc.gpsimd.memset(PP[:], 0.0)
            outbuf = bigpool.tile([BL, TY * VOUT], bf16, name="outbuf",
                                  tag="outbuf")
            ident = cpool.tile([128, 128], bf16, name="ident", tag="ident")
            make_identity(nc, ident)

            # ---- load weights ----
            def ld(pool, dram, shape, dt, tag):
                t = pool.tile(shape, dt, name=tag, tag=tag)
                nc.sync.dma_start(t[:], dram[:])
                return t

            wx_sb = {d: ld(cpool, wx[d], [VIN + 1, 4 * NA], bf16, f"wx{d}")
                     for d in ("f", "b")}
            wh_sb = {d: ld(cpool, wh[d], [NA, 4 * NA], bf16, f"wh{d}")
                     for d in ("f", "b")}
            w1pp = ld(cpool, w1pp_d, [NA, 2 * K1], bf16, "w1pp")
            w1a = ld(cpool, w1a_d, [NS, K1], bf16, "w1a")
            b1r = ld(cpool, b1r_d, [BL, K1], f32, "b1r")
            w2r = ld(cpool, w2r_d, [BL, K1], bf16, "w2r")
            wihp = ld(cpool, wihp_d, [NS, 4 * NS], bf16, "wihp")
            whhp = ld(cpool, whhp_d, [NS, 4 * NS], bf16, "whhp")
            bpr = ld(cpool, bpr_d, [BL, 4 * NS], f32, "bpr")
            w3t = ld(cpool, w3t_d, [NS, VOUT], bf16, "w3t")
            b3r = ld(cpool, b3r_d, [BL, VOUT], f32, "b3r")
            b2s = ld(cpool, b2s_d, [BL, 1], f32, "b2s")

            # ---------------- encoder ----------------
            with (
                tc.tile_pool(name="xbuf", bufs=1) as xpool,
                tc.tile_pool(name="work", bufs=2) as wkpool,
                tc.tile_pool(name="hout", bufs=8) as hpool,
                tc.tile_pool(name="zps", bufs=3, space="PSUM") as zpsum,
                tc.tile_pool(name="pps", bufs=2, space="PSUM") as ppsum,
                tc.tile_pool(name="tps", bufs=2, space="PSUM") as tpsum,
            ):
                xbuf = {}
                for d in ("f", "b"):
                    for s in (0, 1):
                        t = xpool.tile([VIN + 1, TC, BL], fp8,
                                       name=f"x{d}{s}", tag=f"x{d}{s}")
                        nc.gpsimd.memset(t[VIN:VIN + 1, :, :], 1.0)
                        xbuf[d, s] = t

                h0 = cpool.tile([NA, BL], bf16, name="h0", tag="h0")
                nc.gpsimd.memset(h0[:], 0.0)
                cstate = {}
                for d in ("f", "b"):
                    cstate[d] = cpool.tile([NA, BL], f32, name=f"c{d}",
                                           tag=f"c{d}")
                    nc.gpsimd.memset(cstate[d][:], 0.0)
                hprev = {"f": h0, "b": h0}

                for c in range(NCHUNK):
                    nc.sync.dma_start(
                        xbuf["f", c % 2][0:VIN, :, :],
                        xt[TC * c:TC * (c + 1), :, :].rearrange(
                            "t v b -> v t b"))
                    nc.sync.dma_start(
                        xbuf["b", c % 2][0:VIN, :, :],
                        xt[TX - TC * (c + 1):TX - TC * c, :, :].rearrange(
                            "t v b -> v t b"))
                    for tl in range(TC):
                        for di, d in enumerate(("f", "b")):
                            if d == "f":
                                t_act = TC * c + tl
                                xcol = tl
                            else:
                                t_act = TX - 1 - (TC * c + tl)
                                xcol = TC - 1 - tl
                            xrhs = xbuf[d, c % 2][:, xcol, :]
                            z = zpsum.tile([NA, 4 * BL], f32, name="z", tag="z")
                            for g in range(4):
                                cs = slice(g * BL, (g + 1) * BL)
                                ws = slice(g * NA, (g + 1) * NA)
                                nc.tensor.matmul(z[:, cs], wx_sb[d][:, ws],
                                                 xrhs, start=True, stop=False)
                                nc.tensor.matmul(z[:, cs], wh_sb[d][:, ws],
                                                 hprev[d][:], start=False,
                                                 stop=True)
                            T = wkpool.tile([NA, 4 * BL], f32, name="T", tag="T")
                            nc.scalar.activation(T[:], z[:], TH)
                            ti = T[:, 0:BL]
                            tf = T[:, BL:2 * BL]
                            tg = T[:, 2 * BL:3 * BL]
                            to = T[:, 3 * BL:4 * BL]
                            m1 = wkpool.tile([NA, BL], f32, name="m1", tag="m1")
                            m2 = wkpool.tile([NA, BL], f32, name="m2", tag="m2")
                            # C' = 2c; m1=(tf+1)*C'; m2=(ti+1)*tg;
                            # C'new = 0.5*m1 + m2
                            nc.vector.scalar_tensor_tensor(
                                m1[:], tf, 1.0, cstate[d][:], AD, MU)
                            nc.vector.scalar_tensor_tensor(
                                m2[:], ti, 1.0, tg, AD, MU)
                            nc.vector.scalar_tensor_tensor(
                                cstate[d][:], m1[:], 0.5, m2[:], MU, AD)
                            tcell = wkpool.tile([NA, BL], f32, name="tc",
                                                tag="tc")
                            nc.scalar.activation(tcell[:], cstate[d][:], TH,
                                                 scale=0.5)
                            hnew = hpool.tile([NA, BL], bf16, name="h", tag="h")
                            # H' = (to+1)*tanh(c) = 2h
                            nc.vector.scalar_tensor_tensor(
                                hnew[:], to, 1.0, tcell[:], AD, MU)
                            hprev[d] = hnew
                            # PP[:, t, :] += hT @ w1pp_d  (PE, N=10)
                            pp_ps = ppsum.tile([BL, K1], f32, name="pp",
                                               tag="pp")
                            nc.tensor.matmul(
                                pp_ps[:], hnew[:],
                                w1pp[:, di * K1:(di + 1) * K1],
                                start=True, stop=True)
                            nc.vector.tensor_tensor(
                                PP[:, t_act, :], pp_ps[:], PP[:, t_act, :], AD)
                            # preB[:, d*64:(d+1)*64, t] = hnew.T
                            hT = tpsum.tile([BL, NA], bf16, name="hT", tag="hT")
                            nc.tensor.transpose(hT[:], hnew[:],
                                                ident[0:NA, 0:NA])
                            nc.scalar.copy(
                                preB[:, di * NA:(di + 1) * NA, t_act], hT[:])

            # ---------------- decoder ----------------
            with (
                tc.tile_pool(name="dwork", bufs=1) as dpool,
                tc.tile_pool(name="ebuf", bufs=1) as epool,
                tc.tile_pool(name="prod", bufs=2) as prpool,
                tc.tile_pool(name="dz", bufs=2, space="PSUM") as dzps,
                tc.tile_pool(name="dsm", bufs=1, space="PSUM") as dsps,
                tc.tile_pool(name="dtr", bufs=1, space="PSUM") as dtps,
            ):
                s_fT = cpool.tile([NS, BL], bf16, name="sfT", tag="sfT")
                nc.gpsimd.memset(s_fT[:], 0.0)
                cdec = cpool.tile([BL, NS], f32, name="cdec", tag="cdec")
                nc.gpsimd.memset(cdec[:], 0.0)
                e = epool.tile([BL, TX, K1], bf16, name="e", tag="e")

                for t in range(TY):
                    # PS = s'@w1a + b1
                    ps_ps = dsps.tile([BL, K1], f32, name="ps", tag="ps")
                    nc.tensor.matmul(ps_ps[:], s_fT[:], w1a[:],
                                     start=True, stop=True)
                    PS = dpool.tile([BL, K1], f32, name="PS", tag="PS")
                    nc.vector.tensor_tensor(PS[:], ps_ps[:], b1r[:], AD)
                    # e = tanh(PP + PS)
                    nc.vector.tensor_tensor(
                        e[:], PP[:],
                        PS[:, None, :].broadcast_to([BL, TX, K1]), AD)
                    nc.scalar.activation(e[:], e[:], TH)
                    # q = sum_k e*W2 ; u = relu(q + b2); w = exp(u), Z
                    nc.vector.tensor_tensor(
                        e[:], e[:],
                        w2r[:, None, :].broadcast_to([BL, TX, K1]), MU)
                    q = dpool.tile([BL, TX], f32, name="q", tag="q")
                    nc.vector.tensor_reduce(q[:], e[:], AX, AD)
                    u = dpool.tile([BL, TX], f32, name="u", tag="u")
                    nc.scalar.activation(u[:], q[:], RL, bias=b2s)
                    wat = dpool.tile([BL, TX], bf16, name="wat", tag="wat")
                    Z = dpool.tile([BL, 1], f32, name="Z", tag="Z")
                    nc.scalar.activation(wat[:], u[:], EX, accum_out=Z[:])
                    # ctx_u[f] = sum_tx w*preB
                    ctx_u = dpool.tile([BL, NS], f32, name="ctxu", tag="ctxu")
                    for fc in range(NS // FC):
                        fs = slice(fc * FC, (fc + 1) * FC)
                        prod = prpool.tile([BL, FC, TX], bf16, name="prod",
                                           tag="prod")
                        nc.vector.tensor_tensor(
                            prod[:], preB[:, fs, :],
                            wat[:, None, :].broadcast_to([BL, FC, TX]), MU)
                        nc.vector.tensor_reduce(ctx_u[:, fs], prod[:], AX, AD)
                    # ctx = ctx_u / Z  (ACT copy w/ per-partition scale)
                    Zr = dpool.tile([BL, 1], f32, name="Zr", tag="Zr")
                    nc.vector.reciprocal(Zr[:], Z[:])
                    ctx = dpool.tile([BL, NS], bf16, name="ctx", tag="ctx")
                    nc.scalar.activation(ctx[:], ctx_u[:], CP, scale=Zr)
                    # ctx_fT
                    ctT_ps = dtps.tile([NS, BL], bf16, name="ctT", tag="ctT")
                    nc.tensor.transpose(ctT_ps[:], ctx[:], ident[:])
                    ctx_fT = dpool.tile([NS, BL], bf16, name="cfT", tag="cfT")
                    nc.vector.tensor_copy(ctx_fT[:], ctT_ps[:])
                    # z = ctx@wihp + s@whhp + bp
                    z_ps = dzps.tile([BL, 4 * NS], f32, name="zd", tag="zd")
                    nc.tensor.matmul(z_ps[:], ctx_fT[:], wihp[:],
                                     start=True, stop=False)
                    nc.tensor.matmul(z_ps[:], s_fT[:], whhp[:],
                                     start=False, stop=True)
                    zb = dpool.tile([BL, 4 * NS], f32, name="zb", tag="zb")
                    nc.vector.tensor_tensor(zb[:], z_ps[:], bpr[:], AD)
                    # gates: i,f scale 0.5 tanh; g scale 1; o scale 0.5
                    Tg8 = dpool.tile([BL, 4 * NS], f32, name="Tg8", tag="Tg8")
                    nc.scalar.activation(Tg8[:, 0:2 * NS], zb[:, 0:2 * NS],
                                         TH, scale=0.5)
                    nc.scalar.activation(Tg8[:, 2 * NS:3 * NS],
                                         zb[:, 2 * NS:3 * NS], TH)
                    nc.scalar.activation(Tg8[:, 3 * NS:4 * NS],
                                         zb[:, 3 * NS:4 * NS], TH, scale=0.5)
                    ti = Tg8[:, 0:NS]
                    tf = Tg8[:, NS:2 * NS]
                    tg = Tg8[:, 2 * NS:3 * NS]
                    to = Tg8[:, 3 * NS:4 * NS]
                    m1 = dpool.tile([BL, NS], f32, name="dm1", tag="dm1")
                    m2 = dpool.tile([BL, NS], f32, name="dm2", tag="dm2")
                    nc.vector.scalar_tensor_tensor(m1[:], tf, 1.0, cdec[:],
                                                   AD, MU)
                    nc.vector.scalar_tensor_tensor(m2[:], ti, 1.0, tg, AD, MU)
                    nc.vector.scalar_tensor_tensor(cdec[:], m1[:], 0.5, m2[:],
                                                   MU, AD)
                    tcl = dpool.tile([BL, NS], f32, name="dtc", tag="dtc")
                    nc.scalar.activation(tcl[:], cdec[:], TH, scale=0.5)
                    s_b = dpool.tile([BL, NS], bf16, name="sb", tag="sb")
                    nc.vector.scalar_tensor_tensor(s_b[:], to, 1.0, tcl[:],
                                                   AD, MU)
                    # s_fT = s_b.T
                    sT_ps = dtps.tile([NS, BL], bf16, name="sT", tag="sT")
                    nc.tensor.transpose(sT_ps[:], s_b[:], ident[:])
                    nc.vector.tensor_copy(s_fT[:], sT_ps[:])
                    # logits
                    L_ps = dsps.tile([BL, VOUT], f32, name="L", tag="L")
                    nc.tensor.matmul(L_ps[:], s_fT[:], w3t[:],
                                     start=True, stop=True)
                    nc.vector.tensor_tensor(
                        outbuf[:, t * VOUT:(t + 1) * VOUT], L_ps[:], b3r[:],
                        AD)

                nc.sync.dma_start(logit[:], outbuf[:])

    nc.compile()
    return nc


def _get_nc():
    if "nc" not in _CACHE:
        _CACHE["nc"] = _build()
    return _CACHE["nc"]


def _prep_enc_weights(Wih, Whh, bih, bhh, bf16):
    """Baseline folding: sigmoid-from-tanh 0.5 scales + bias row; Whh gets
    an extra 0.5 because the h it multiplies is stored doubled."""
    b = (bih + bhh).astype(np.float32)
    scale = np.concatenate([np.full(2 * NA, 0.5, np.float32),
                            np.full(NA, 1.0, np.float32),
                            np.full(NA, 0.5, np.float32)])
    Wx = (Wih * scale[:, None]).astype(np.float32)
    Wh = (Whh * (0.5 * scale)[:, None]).astype(np.float32)
    bb = (b * scale).astype(np.float32)
    wx_aug = np.concatenate([Wx.T, bb[None, :]], axis=0)
    return (np.ascontiguousarray(wx_aug).astype(bf16),
            np.ascontiguousarray(Wh.T).astype(bf16))


def _run_cached(nc, in_maps):
    import jax
    import numpy as _np
    from jax.sharding import Mesh, PartitionSpec
    from jax.experimental.shard_map import shard_map
    from concourse import bass2jax as b2j

    if "runner" not in _CACHE:
        b2j.install_neuronx_cc_hook()
        import concourse.mybir as mybir
        pname = (nc.partition_id_tensor.name
                 if nc.partition_id_tensor else None)
        in_names, out_names, out_avals = [], [], []
        for alloc in nc.m.functions[0].allocations:
            if not isinstance(alloc, mybir.MemoryLocationSet):
                continue
            name = alloc.memorylocations[0].name
            if alloc.kind == "ExternalInput":
                if name != pname:
                    in_names.append(name)
            elif alloc.kind == "ExternalOutput":
                out_names.append(name)
                out_avals.append(jax.core.ShapedArray(
                    tuple(alloc.tensor_shape), mybir.dt.np(alloc.dtype)))
        n_params = len(in_names)
        all_names = in_names + out_names
        if pname is not None:
            all_names = all_names + [pname]

        def _body(*args):
            ops = list(args)
            if pname is not None:
                ops.append(b2j.partition_id_tensor())
            outs = b2j._bass_exec_p.bind(
                *ops, out_avals=tuple(out_avals), in_names=tuple(all_names),
                out_names=tuple(out_names), lowering_input_output_aliases=(),
                sim_require_finite=True, sim_require_nnan=True, nc=nc)
            return tuple(outs)

        devices = jax.devices()[:NCORES]
        mesh = Mesh(_np.asarray(devices), ("core",))
        nio = n_params + len(out_names)
        sharded = jax.jit(
            shard_map(_body, mesh=mesh,
                      in_specs=(PartitionSpec("core"),) * nio,
                      out_specs=(PartitionSpec("core"),) * len(out_names),
                      check_rep=False),
            donate_argnums=tuple(range(n_params, nio)), keep_unused=True)
        _CACHE["runner"] = (sharded, in_names, out_names, out_avals, n_params)

    sharded, in_names, out_names, out_avals, n_params = _CACHE["runner"]
    concat_in = [_np.concatenate([_np.asarray(m[n]) for m in in_maps], axis=0)
                 for n in in_names]
    concat_zeros = [
        _np.zeros((NCORES * a.shape[0], *a.shape[1:]), a.dtype)
        for a in out_avals]
    out_arrs = sharded(*concat_in, *concat_zeros)
    return [
        {n: _np.asarray(out_arrs[i]).reshape(NCORES, *out_avals[i].shape)[c]
         for i, n in enumerate(out_names)}
        for c in range(NCORES)
    ]


def kernel(X, Wih_f, Whh_f, bih_f, bhh_f, Wih_b, Whh_b, bih_b, bhh_b,
           Wih_p, Whh_p, bih_p, bhh_p, W1, b1, W2, b2, W3, b3):
    import ml_dtypes
    bf16 = ml_dtypes.bfloat16
    fp8 = ml_dtypes.float8_e3m4

    _t = {}
    _t0 = _time.time()
    nc = _get_nc()
    _t['build'] = _time.time() - _t0
    _t0 = _time.time()

    wf = _prep_enc_weights(Wih_f, Whh_f, bih_f, bhh_f, bf16)
    wb = _prep_enc_weights(Wih_b, Whh_b, bih_b, bhh_b, bf16)

    W1 = np.asarray(W1, np.float32)
    # decoder weights (shared across cores). All "0.5" folds: the doubled
    # h/s/pre states.
    w1pp = np.concatenate([0.5 * W1[:, NS:NS + NA].T,
                           0.5 * W1[:, NS + NA:2 * NS].T],
                          axis=1).astype(bf16)                   # (64, 20)
    w1a = np.ascontiguousarray(0.5 * W1[:, :NS].T).astype(bf16)  # (128, 10)
    b1r = np.broadcast_to(np.asarray(b1, np.float32), (BL, K1)).copy()
    w2r = np.broadcast_to(np.asarray(W2[0], np.float32).astype(bf16),
                          (BL, K1)).copy()
    wihp = np.ascontiguousarray(0.5 * np.asarray(Wih_p, np.float32).T
                                ).astype(bf16)                   # (128, 512)
    whhp = np.ascontiguousarray(0.5 * np.asarray(Whh_p, np.float32).T
                                ).astype(bf16)
    bpr = np.broadcast_to((np.asarray(bih_p, np.float32)
                           + np.asarray(bhh_p, np.float32)), (BL, 4 * NS)
                          ).copy()
    w3t = np.ascontiguousarray(0.5 * np.asarray(W3, np.float32).T
                               ).astype(bf16)                    # (128, 32)
    b3r = np.broadcast_to(np.asarray(b3, np.float32), (BL, VOUT)).copy()
    b2s = np.full((BL, 1), float(np.asarray(b2).ravel()[0]), np.float32)

    Xb = np.asarray(X)
    in_maps = []
    for c in range(NCORES):
        xc = Xb[c * BL:(c + 1) * BL]                     # (128, 512, 64)
        xtc = np.ascontiguousarray(
            xc.transpose(1, 2, 0)).astype(fp8)           # (512, 64, 128)
        m = {"xt": xtc, "w1pp": w1pp, "w1a": w1a, "b1r": b1r, "w2r": w2r,
             "wihp": wihp, "whhp": whhp, "bpr": bpr, "w3t": w3t,
             "b3r": b3r, "b2s": b2s}
        for d, w in (("f", wf), ("b", wb)):
            m[f"wx{d}"] = w[0]
            m[f"wh{d}"] = w[1]
        in_maps.append(m)

    _t['prep'] = _time.time() - _t0
    _t0 = _time.time()
    try:
        results = _run_cached(nc, in_maps)
    except Exception:
        from concourse.bass_utils import run_bass_kernel_spmd
        results = run_bass_kernel_spmd(
            nc, in_maps, core_ids=list(range(NCORES))).results
    _t['spmd'] = _time.time() - _t0
    _t0 = _time.time()
    _CACHE["last_results"] = results
    _CACHE["last_in_maps"] = in_maps

    # logits (B, TY, VOUT); final softmax over batch axis
    L = np.concatenate(
        [results[c]["logit"].reshape(BL, TY, VOUT).astype(np.float32)
         for c in range(NCORES)], axis=0)
    L -= L.max(axis=0, keepdims=True)
    np.exp(L, out=L)
    L /= L.sum(axis=0, keepdims=True)
    _t['post'] = _time.time() - _t0
    _CACHE['timers'] = _t
    return np.ascontiguousarray(L)


# revision 4
# speedup vs baseline: 1.9604x; 1.0346x over previous
"""Trainium2 Bass kernel for nn_DateParser — fully fused on-device.

Data-parallel over batch: 1024 -> 8 cores x 128. The axon tunnel moves
~50 MB/s, so the whole model (bidirectional LSTM encoder + attention
decoder) runs on-device; only X (fp8-e3m4, 34 MB) goes up and bf16
logits (2 MB) come back. Host does only an fp8 cast on the way in and
the final softmax over the batch axis (it spans all cores).

X ships in natural [batch, time, vin] layout (no host transpose); the
per-timestep [v, b] operand the gate matmuls need is produced on-device
with PE transposes.

Encoder: transposed layout (gate dim on partitions, batch on free),
sigmoid via tanh identity with scales/biases folded into weights
(one activation table set: exp_and_others covers tanh/exp/relu/copy).
h is stored doubled (H' = 2h) so sigma(x) = (1+tanh(x/2))/2 needs no
extra multiply; all downstream weights absorb the 0.5.

Decoder (on device, batch-on-partitions layout): per step
  PS = s'@W1a' + b1                  (PE; W1a' pre-halved)
  e = tanh(PP + PS)                  (DVE add w/ bcast + ACT tanh)
  q = sum_k e*W2                     (DVE mult w/ bcast + reduce)
  w = exp(relu(q + b2)), Z = sum(w)  (ACT with accum)
  ctx = (sum_tx w * preB) / Z        (DVE bf16 mult+reduce, ACT scale)
  LSTM cell on [b, 4NS] gates        (PE matmuls + ACT/DVE pointwise)
  logits -> SBUF outbuf              (PE + DVE)
preB ([b, f, tx] bf16, 16.8MB) and PP ([b, tx, 10]) are built during
the encoder via PE transposes of the h tiles, so pre never leaves the
device.
"""

import numpy as np
import time as _time

B, TX, TY = 1024, 512, 32
NA, NS = 64, 128
VIN, VOUT = 64, 32
K1 = 10
NCORES = 8
BL = B // NCORES          # 128 batch per core
TC = 16                   # time-chunk for X streaming
NCHUNK = TX // TC
FC = 4                    # feature-chunk for ctx reduce

_CACHE = {}


def _build(unused=None):
    import concourse.bass as bass
    import concourse.bacc as bacc
    import concourse.mybir as mybir
    from concourse import tile
    from concourse.masks import make_identity

    nc = bacc.Bacc("TRN2", target_bir_lowering=False, debug=False,
                   num_devices=NCORES)
    f32 = mybir.dt.float32
    bf16 = mybir.dt.bfloat16
    fp8 = mybir.dt.float8e3
    TH = mybir.ActivationFunctionType.Tanh
    EX = mybir.ActivationFunctionType.Exp
    RL = mybir.ActivationFunctionType.Relu
    CP = mybir.ActivationFunctionType.Copy
    AD, MU = mybir.AluOpType.add, mybir.AluOpType.mult
    AX = mybir.AxisListType.X

    xt = nc.dram_tensor("xt", [BL, TX, VIN], fp8, kind="ExternalInput").ap()
    wx = {}
    wh = {}
    for d in ("f", "b"):
        wx[d] = nc.dram_tensor(f"wx{d}", [VIN + 1, 4 * NA], bf16,
                               kind="ExternalInput").ap()
        wh[d] = nc.dram_tensor(f"wh{d}", [NA, 4 * NA], bf16,
                               kind="ExternalInput").ap()
    w1pp_d = nc.dram_tensor("w1pp", [NA, 2 * K1], bf16, kind="ExternalInput").ap()
    w1a_d = nc.dram_tensor("w1a", [NS, K1], bf16, kind="ExternalInput").ap()
    b1r_d = nc.dram_tensor("b1r", [BL, K1], f32, kind="ExternalInput").ap()
    w2r_d = nc.dram_tensor("w2r", [BL, K1], bf16, kind="ExternalInput").ap()
    wihp_d = nc.dram_tensor("wihp", [NS, 4 * NS], bf16, kind="ExternalInput").ap()
    whhp_d = nc.dram_tensor("whhp", [NS, 4 * NS], bf16, kind="ExternalInput").ap()
    bpr_d = nc.dram_tensor("bpr", [BL, 4 * NS], f32, kind="ExternalInput").ap()
    w3t_d = nc.dram_tensor("w3t", [NS, VOUT], bf16, kind="ExternalInput").ap()
    b3r_d = nc.dram_tensor("b3r", [BL, VOUT], f32, kind="ExternalInput").ap()
    b2s_d = nc.dram_tensor("b2s", [BL, 1], f32, kind="ExternalInput").ap()

    logit = nc.dram_tensor("logit", [BL, TY * VOUT], bf16,
                           kind="ExternalOutput").ap()

    with tile.TileContext(nc) as tc:
        with (
            tc.tile_pool(name="const", bufs=1) as cpool,
            tc.tile_pool(name="big", bufs=1) as bigpool,
        ):
            # ---- persistent SBUF tensors ----
            preB = bigpool.tile([BL, NS, TX], bf16, name="preB", tag="preB")
            PP = bigpool.tile([BL, TX, K1], bf16, name="PP", tag="PP")
            nc.gpsimd.memset(PP[:], 0.0)
            outbuf = bigpool.tile([BL, TY * VOUT], bf16, name="outbuf",
                                  tag="outbuf")
            ident = cpool.tile([128, 128], bf16, name="ident", tag="ident")
            make_identity(nc, ident)

            # ---- load weights ----
            def ld(pool, dram, shape, dt, tag):
                t = pool.tile(shape, dt, name=tag, tag=tag)
                nc.sync.dma_start(t[:], dram[:])
                return t

            wx_sb = {d: ld(cpool, wx[d], [VIN + 1, 4 * NA], bf16, f"wx{d}")
                     for d in ("f", "b")}
            wh_sb = {d: ld(cpool, wh[d], [NA, 4 * NA], bf16, f"wh{d}")
                     for d in ("f", "b")}
            w1pp = ld(cpool, w1pp_d, [NA, 2 * K1], bf16, "w1pp")
            w1a = ld(cpool, w1a_d, [NS, K1], bf16, "w1a")
            b1r = ld(cpool, b1r_d, [BL, K1], f32, "b1r")
            w2r = ld(cpool, w2r_d, [BL, K1], bf16, "w2r")
            wihp = ld(cpool, wihp_d, [NS, 4 * NS], bf16, "wihp")
            whhp = ld(cpool, whhp_d, [NS, 4 * NS], bf16, "whhp")
            bpr = ld(cpool, bpr_d, [BL, 4 * NS], f32, "bpr")
            w3t = ld(cpool, w3t_d, [NS, VOUT], bf16, "w3t")
            b3r = ld(cpool, b3r_d, [BL, VOUT], f32, "b3r")
            b2s = ld(cpool, b2s_d, [BL, 1], f32, "b2s")

            # ---------------- encoder ----------------
            with (
                tc.tile_pool(name="x8", bufs=1) as x8pool,
                tc.tile_pool(name="xc", bufs=1) as xcpool,
                tc.tile_pool(name="xrow", bufs=1) as xrpool,
                tc.tile_pool(name="work", bufs=2) as wkpool,
                tc.tile_pool(name="hout", bufs=8) as hpool,
                tc.tile_pool(name="zps", bufs=2, space="PSUM") as zpsum,
                tc.tile_pool(name="pps", bufs=2, space="PSUM") as ppsum,
                tc.tile_pool(name="tps", bufs=2, space="PSUM") as tpsum,
                tc.tile_pool(name="xps", bufs=2, space="PSUM") as xpsum,
            ):
                # fp8 landing buffers + bf16 chunk buffers (double-buffered
                # per dir)
                xb8 = {}
                xbc = {}
                for d in ("f", "b"):
                    for s in (0, 1):
                        xb8[d, s] = x8pool.tile([BL, TC, VIN], fp8,
                                                name=f"x8{d}{s}",
                                                tag=f"x8{d}{s}")
                        xbc[d, s] = xcpool.tile([BL, TC, VIN], bf16,
                                                name=f"xc{d}{s}",
                                                tag=f"xc{d}{s}")
                # rotating [v+1, b] matmul operands, ones row preset
                xrow = []
                for i in range(4):
                    t = xrpool.tile([VIN + 1, BL], bf16, name=f"xr{i}",
                                    tag=f"xr{i}")
                    nc.gpsimd.memset(t[VIN:VIN + 1, :], 1.0)
                    xrow.append(t)
                xri = 0

                h0 = cpool.tile([NA, BL], bf16, name="h0", tag="h0")
                nc.gpsimd.memset(h0[:], 0.0)
                cstate = {}
                for d in ("f", "b"):
                    cstate[d] = cpool.tile([NA, BL], f32, name=f"c{d}",
                                           tag=f"c{d}")
                    nc.gpsimd.memset(cstate[d][:], 0.0)
                hprev = {"f": h0, "b": h0}

                for c in range(NCHUNK):
                    for d in ("f", "b"):
                        if d == "f":
                            ts0 = TC * c
                        else:
                            ts0 = TX - TC * (c + 1)
                        nc.sync.dma_start(xb8[d, c % 2][:],
                                          xt[:, ts0:ts0 + TC, :])
                        nc.vector.tensor_copy(xbc[d, c % 2][:],
                                              xb8[d, c % 2][:])
                    for tl in range(TC):
                        for di, d in enumerate(("f", "b")):
                            if d == "f":
                                t_act = TC * c + tl
                                xcol = tl
                            else:
                                t_act = TX - 1 - (TC * c + tl)
                                xcol = TC - 1 - tl
                            # x_t [v, b] via PE transpose
                            xT = xpsum.tile([VIN, BL], bf16, name="xT",
                                            tag="xT")
                            nc.tensor.transpose(
                                xT[:], xbc[d, c % 2][:, xcol, :], ident[:])
                            xr = xrow[xri % 4]
                            xri += 1
                            nc.scalar.copy(xr[0:VIN, :], xT[:])
                            z = zpsum.tile([NA, 4 * BL], f32, name="z", tag="z")
                            for g in range(4):
                                cs = slice(g * BL, (g + 1) * BL)
                                ws = slice(g * NA, (g + 1) * NA)
                                nc.tensor.matmul(z[:, cs], wx_sb[d][:, ws],
                                                 xr[:], start=True, stop=False)
                                nc.tensor.matmul(z[:, cs], wh_sb[d][:, ws],
                                                 hprev[d][:], start=False,
                                                 stop=True)
                            T = wkpool.tile([NA, 4 * BL], f32, name="T", tag="T")
                            nc.scalar.activation(T[:], z[:], TH)
                            ti = T[:, 0:BL]
                            tf = T[:, BL:2 * BL]
                            tg = T[:, 2 * BL:3 * BL]
                            to = T[:, 3 * BL:4 * BL]
                            m1 = wkpool.tile([NA, BL], f32, name="m1", tag="m1")
                            m2 = wkpool.tile([NA, BL], f32, name="m2", tag="m2")
                            # C' = 2c; m1=(tf+1)*C'; m2=(ti+1)*tg;
                            # C'new = 0.5*m1 + m2
                            nc.vector.scalar_tensor_tensor(
                                m1[:], tf, 1.0, cstate[d][:], AD, MU)
                            nc.vector.scalar_tensor_tensor(
                                m2[:], ti, 1.0, tg, AD, MU)
                            nc.vector.scalar_tensor_tensor(
                                cstate[d][:], m1[:], 0.5, m2[:], MU, AD)
                            tcell = wkpool.tile([NA, BL], f32, name="tc",
                                                tag="tc")
                            nc.scalar.activation(tcell[:], cstate[d][:], TH,
                                                 scale=0.5)
                            hnew = hpool.tile([NA, BL], bf16, name="h", tag="h")
                            # H' = (to+1)*tanh(c) = 2h
                            nc.vector.scalar_tensor_tensor(
                                hnew[:], to, 1.0, tcell[:], AD, MU)
                            hprev[d] = hnew
                            # PP[:, t, :] += hT @ w1pp_d  (PE, N=10)
                            pp_ps = ppsum.tile([BL, K1], f32, name="pp",
                                               tag="pp")
                            nc.tensor.matmul(
                                pp_ps[:], hnew[:],
                                w1pp[:, di * K1:(di + 1) * K1],
                                start=True, stop=True)
                            nc.vector.tensor_tensor(
                                PP[:, t_act, :], pp_ps[:], PP[:, t_act, :], AD)
                            # preB[:, d*64:(d+1)*64, t] = hnew.T
                            hT = tpsum.tile([BL, NA], bf16, name="hT", tag="hT")
                            nc.tensor.transpose(hT[:], hnew[:],
                                                ident[0:NA, 0:NA])
                            nc.scalar.copy(
                                preB[:, di * NA:(di + 1) * NA, t_act], hT[:])

            # ---------------- decoder ----------------
            with (
                tc.tile_pool(name="dwork", bufs=1) as dpool,
                tc.tile_pool(name="ebuf", bufs=1) as epool,
                tc.tile_pool(name="prod", bufs=2) as prpool,
                tc.tile_pool(name="dz", bufs=2, space="PSUM") as dzps,
                tc.tile_pool(name="dsm", bufs=1, space="PSUM") as dsps,
                tc.tile_pool(name="dtr", bufs=1, space="PSUM") as dtps,
            ):
                s_fT = cpool.tile([NS, BL], bf16, name="sfT", tag="sfT")
                nc.gpsimd.memset(s_fT[:], 0.0)
                cdec = cpool.tile([BL, NS], f32, name="cdec", tag="cdec")
                nc.gpsimd.memset(cdec[:], 0.0)
                e = epool.tile([BL, TX, K1], bf16, name="e", tag="e")

                for t in range(TY):
                    # PS = s'@w1a + b1
                    ps_ps = dsps.tile([BL, K1], f32, name="ps", tag="ps")
                    nc.tensor.matmul(ps_ps[:], s_fT[:], w1a[:],
                                     start=True, stop=True)
                    PS = dpool.tile([BL, K1], f32, name="PS", tag="PS")
                    nc.vector.tensor_tensor(PS[:], ps_ps[:], b1r[:], AD)
                    # e = tanh(PP + PS)
                    nc.vector.tensor_tensor(
                        e[:], PP[:],
                        PS[:, None, :].broadcast_to([BL, TX, K1]), AD)
                    nc.scalar.activation(e[:], e[:], TH)
                    # q = sum_k e*W2 ; u = relu(q + b2); w = exp(u), Z
                    nc.vector.tensor_tensor(
                        e[:], e[:],
                        w2r[:, None, :].broadcast_to([BL, TX, K1]), MU)
                    q = dpool.tile([BL, TX], f32, name="q", tag="q")
                    nc.vector.tensor_reduce(q[:], e[:], AX, AD)
                    u = dpool.tile([BL, TX], f32, name="u", tag="u")
                    nc.scalar.activation(u[:], q[:], RL, bias=b2s)
                    wat = dpool.tile([BL, TX], bf16, name="wat", tag="wat")
                    Z = dpool.tile([BL, 1], f32, name="Z", tag="Z")
                    nc.scalar.activation(wat[:], u[:], EX, accum_out=Z[:])
                    # ctx_u[f] = sum_tx w*preB
                    ctx_u = dpool.tile([BL, NS], f32, name="ctxu", tag="ctxu")
                    for fc in range(NS // FC):
                        fs = slice(fc * FC, (fc + 1) * FC)
                        prod = prpool.tile([BL, FC, TX], bf16, name="prod",
                                           tag="prod")
                        nc.vector.tensor_tensor(
                            prod[:], preB[:, fs, :],
                            wat[:, None, :].broadcast_to([BL, FC, TX]), MU)
                        nc.vector.tensor_reduce(ctx_u[:, fs], prod[:], AX, AD)
                    # ctx = ctx_u / Z  (ACT copy w/ per-partition scale)
                    Zr = dpool.tile([BL, 1], f32, name="Zr", tag="Zr")
                    nc.vector.reciprocal(Zr[:], Z[:])
                    ctx = dpool.tile([BL, NS], bf16, name="ctx", tag="ctx")
                    nc.scalar.activation(ctx[:], ctx_u[:], CP, scale=Zr)
                    # ctx_fT
                    ctT_ps = dtps.tile([NS, BL], bf16, name="ctT", tag="ctT")
                    nc.tensor.transpose(ctT_ps[:], ctx[:], ident[:])
                    ctx_fT = dpool.tile([NS, BL], bf16, name="cfT", tag="cfT")
                    nc.vector.tensor_copy(ctx_fT[:], ctT_ps[:])
                    # z = ctx@wihp + s@whhp + bp
                    z_ps = dzps.tile([BL, 4 * NS], f32, name="zd", tag="zd")
                    nc.tensor.matmul(z_ps[:], ctx_fT[:], wihp[:],
                                     start=True, stop=False)
                    nc.tensor.matmul(z_ps[:], s_fT[:], whhp[:],
                                     start=False, stop=True)
                    zb = dpool.tile([BL, 4 * NS], f32, name="zb", tag="zb")
                    nc.vector.tensor_tensor(zb[:], z_ps[:], bpr[:], AD)
                    # gates: i,f scale 0.5 tanh; g scale 1; o scale 0.5
                    Tg8 = dpool.tile([BL, 4 * NS], f32, name="Tg8", tag="Tg8")
                    nc.scalar.activation(Tg8[:, 0:2 * NS], zb[:, 0:2 * NS],
                                         TH, scale=0.5)
                    nc.scalar.activation(Tg8[:, 2 * NS:3 * NS],
                                         zb[:, 2 * NS:3 * NS], TH)
                    nc.scalar.activation(Tg8[:, 3 * NS:4 * NS],
                                         zb[:, 3 * NS:4 * NS], TH, scale=0.5)
                    ti = Tg8[:, 0:NS]
                    tf = Tg8[:, NS:2 * NS]
                    tg = Tg8[:, 2 * NS:3 * NS]
                    to = Tg8[:, 3 * NS:4 * NS]
                    m1 = dpool.tile([BL, NS], f32, name="dm1", tag="dm1")
                    m2 = dpool.tile([BL, NS], f32, name="dm2", tag="dm2")
                    nc.vector.scalar_tensor_tensor(m1[:], tf, 1.0, cdec[:],
                                                   AD, MU)
                    nc.vector.scalar_tensor_tensor(m2[:], ti, 1.0, tg, AD, MU)
                    nc.vector.scalar_tensor_tensor(cdec[:], m1[:], 0.5, m2[:],
                                                   MU, AD)
                    tcl = dpool.tile([BL, NS], f32, name="dtc", tag="dtc")
                    nc.scalar.activation(tcl[:], cdec[:], TH, scale=0.5)
                    s_b = dpool.tile([BL, NS], bf16, name="sb", tag="sb")
                    nc.vector.scalar_tensor_tensor(s_b[:], to, 1.0, tcl[:],
                                                   AD, MU)
                    # s_fT = s_b.T
                    sT_ps = dtps.tile([NS, BL], bf16, name="sT", tag="sT")
                    nc.tensor.transpose(sT_ps[:], s_b[:], ident[:])
                    nc.vector.tensor_copy(s_fT[:], sT_ps[:])
                    # logits
                    L_ps = dsps.tile([BL, VOUT], f32, name="L", tag="L")
                    nc.tensor.matmul(L_ps[:], s_fT[:], w3t[:],
                                     start=True, stop=True)
                    nc.vector.tensor_tensor(
                        outbuf[:, t * VOUT:(t + 1) * VOUT], L_ps[:], b3r[:],
                        AD)

                nc.sync.dma_start(logit[:], outbuf[:])

    nc.compile()
    return nc


def _get_nc():
    if "nc" not in _CACHE:
        _CACHE["nc"] = _build()
    return _CACHE["nc"]


def _prep_enc_weights(Wih, Whh, bih, bhh, bf16):
    """Baseline folding: sigmoid-from-tanh 0.5 scales + bias row; Whh gets
    an extra 0.5 because the h it multiplies is stored doubled."""
    b = (bih + bhh).astype(np.float32)
    scale = np.concatenate([np.full(2 * NA, 0.5, np.float32),
                            np.full(NA, 1.0, np.float32),
                            np.full(NA, 0.5, np.float32)])
    Wx = (Wih * scale[:, None]).astype(np.float32)
    Wh = (Whh * (0.5 * scale)[:, None]).astype(np.float32)
    bb = (b * scale).astype(np.float32)
    wx_aug = np.concatenate([Wx.T, bb[None, :]], axis=0)
    return (np.ascontiguousarray(wx_aug).astype(bf16),
            np.ascontiguousarray(Wh.T).astype(bf16))


def _run_cached(nc, in_maps):
    import jax
    import numpy as _np
    from jax.sharding import Mesh, PartitionSpec
    from jax.experimental.shard_map import shard_map
    from concourse import bass2jax as b2j

    if "runner" not in _CACHE:
        b2j.install_neuronx_cc_hook()
        import concourse.mybir as mybir
        pname = (nc.partition_id_tensor.name
                 if nc.partition_id_tensor else None)
        in_names, out_names, out_avals = [], [], []
        for alloc in nc.m.functions[0].allocations:
            if not isinstance(alloc, mybir.MemoryLocationSet):
                continue
            name = alloc.memorylocations[0].name
            if alloc.kind == "ExternalInput":
                if name != pname:
                    in_names.append(name)
            elif alloc.kind == "ExternalOutput":
                out_names.append(name)
                out_avals.append(jax.core.ShapedArray(
                    tuple(alloc.tensor_shape), mybir.dt.np(alloc.dtype)))
        n_params = len(in_names)
        all_names = in_names + out_names
        if pname is not None:
            all_names = all_names + [pname]

        def _body(*args):
            ops = list(args)
            if pname is not None:
                ops.append(b2j.partition_id_tensor())
            outs = b2j._bass_exec_p.bind(
                *ops, out_avals=tuple(out_avals), in_names=tuple(all_names),
                out_names=tuple(out_names), lowering_input_output_aliases=(),
                sim_require_finite=True, sim_require_nnan=True, nc=nc)
            return tuple(outs)

        devices = jax.devices()[:NCORES]
        mesh = Mesh(_np.asarray(devices), ("core",))
        nio = n_params + len(out_names)
        sharded = jax.jit(
            shard_map(_body, mesh=mesh,
                      in_specs=(PartitionSpec("core"),) * nio,
                      out_specs=(PartitionSpec("core"),) * len(out_names),
                      check_rep=False),
            donate_argnums=tuple(range(n_params, nio)), keep_unused=True)
        _CACHE["runner"] = (sharded, in_names, out_names, out_avals, n_params)

    sharded, in_names, out_names, out_avals, n_params = _CACHE["runner"]
    concat_in = [_np.concatenate([_np.asarray(m[n]) for m in in_maps], axis=0)
                 for n in in_names]
    concat_zeros = [
        _np.zeros((NCORES * a.shape[0], *a.shape[1:]), a.dtype)
        for a in out_avals]
    out_arrs = sharded(*concat_in, *concat_zeros)
    return [
        {n: _np.asarray(out_arrs[i]).reshape(NCORES, *out_avals[i].shape)[c]
         for i, n in enumerate(out_names)}
        for c in range(NCORES)
    ]


def kernel(X, Wih_f, Whh_f, bih_f, bhh_f, Wih_b, Whh_b, bih_b, bhh_b,
           Wih_p, Whh_p, bih_p, bhh_p, W1, b1, W2, b2, W3, b3):
    import ml_dtypes
    bf16 = ml_dtypes.bfloat16
    fp8 = ml_dtypes.float8_e3m4

    _t = {}
    _t0 = _time.time()
    nc = _get_nc()
    _t['build'] = _time.time() - _t0
    _t0 = _time.time()

    wf = _prep_enc_weights(Wih_f, Whh_f, bih_f, bhh_f, bf16)
    wb = _prep_enc_weights(Wih_b, Whh_b, bih_b, bhh_b, bf16)

    W1 = np.asarray(W1, np.float32)
    # decoder weights (shared across cores). All "0.5" folds: the doubled
    # h/s/pre states.
    w1pp = np.concatenate([0.5 * W1[:, NS:NS + NA].T,
                           0.5 * W1[:, NS + NA:2 * NS].T],
                          axis=1).astype(bf16)                   # (64, 20)
    w1a = np.ascontiguousarray(0.5 * W1[:, :NS].T).astype(bf16)  # (128, 10)
    b1r = np.broadcast_to(np.asarray(b1, np.float32), (BL, K1)).copy()
    w2r = np.broadcast_to(np.asarray(W2[0], np.float32).astype(bf16),
                          (BL, K1)).copy()
    wihp = np.ascontiguousarray(0.5 * np.asarray(Wih_p, np.float32).T
                                ).astype(bf16)                   # (128, 512)
    whhp = np.ascontiguousarray(0.5 * np.asarray(Whh_p, np.float32).T
                                ).astype(bf16)
    bpr = np.broadcast_to((np.asarray(bih_p, np.float32)
                           + np.asarray(bhh_p, np.float32)), (BL, 4 * NS)
                          ).copy()
    w3t = np.ascontiguousarray(0.5 * np.asarray(W3, np.float32).T
                               ).astype(bf16)                    # (128, 32)
    b3r = np.broadcast_to(np.asarray(b3, np.float32), (BL, VOUT)).copy()
    b2s = np.full((BL, 1), float(np.asarray(b2).ravel()[0]), np.float32)

    X8 = np.asarray(X).astype(fp8)                   # (1024, 512, 64) fp8
    in_maps = []
    for c in range(NCORES):
        m = {"xt": X8[c * BL:(c + 1) * BL], "w1pp": w1pp, "w1a": w1a,
             "b1r": b1r, "w2r": w2r, "wihp": wihp, "whhp": whhp, "bpr": bpr,
             "w3t": w3t, "b3r": b3r, "b2s": b2s}
        for d, w in (("f", wf), ("b", wb)):
            m[f"wx{d}"] = w[0]
            m[f"wh{d}"] = w[1]
        in_maps.append(m)

    _t['prep'] = _time.time() - _t0
    _t0 = _time.time()
    try:
        results = _run_cached(nc, in_maps)
    except Exception:
        from concourse.bass_utils import run_bass_kernel_spmd
        results = run_bass_kernel_spmd(
            nc, in_maps, core_ids=list(range(NCORES))).results
    _t['spmd'] = _time.time() - _t0
    _t0 = _time.time()
    _CACHE["last_results"] = results
    _CACHE["last_in_maps"] = in_maps

    # logits (B, TY, VOUT); final softmax over batch axis
    L = np.concatenate(
        [results[c]["logit"].reshape(BL, TY, VOUT).astype(np.float32)
         for c in range(NCORES)], axis=0)
    L -= L.max(axis=0, keepdims=True)
    np.exp(L, out=L)
    L /= L.sum(axis=0, keepdims=True)
    _t['post'] = _time.time() - _t0
    _CACHE['timers'] = _t
    return np.ascontiguousarray(L)


# revision 5
# speedup vs baseline: 1.9891x; 1.0146x over previous
"""Trainium2 Bass kernel for nn_DateParser — fully fused on-device.

Data-parallel over batch: 1024 -> 8 cores x 128. The axon tunnel moves
~50 MB/s, so the whole model (bidirectional LSTM encoder + attention
decoder) runs on-device; only X (fp8-e3m4, 34 MB) goes up and bf16
logits (2 MB) come back. Host does only an fp8 cast on the way in and
the final softmax over the batch axis (it spans all cores).

X ships in natural [batch, time, vin] layout (no host transpose); the
per-timestep [v, b] operand the gate matmuls need is produced on-device
with PE transposes.

Encoder: transposed layout (gate dim on partitions, batch on free),
sigmoid via tanh identity with scales/biases folded into weights
(one activation table set: exp_and_others covers tanh/exp/relu/copy).
h is stored doubled (H' = 2h) so sigma(x) = (1+tanh(x/2))/2 needs no
extra multiply; all downstream weights absorb the 0.5.

Decoder (on device, batch-on-partitions layout): per step
  PS = s'@W1a' + b1                  (PE; W1a' pre-halved)
  e = tanh(PP + PS)                  (DVE add w/ bcast + ACT tanh)
  q = sum_k e*W2                     (DVE mult w/ bcast + reduce)
  w = exp(relu(q + b2)), Z = sum(w)  (ACT with accum)
  ctx = (sum_tx w * preB) / Z        (DVE bf16 mult+reduce, ACT scale)
  LSTM cell on [b, 4NS] gates        (PE matmuls + ACT/DVE pointwise)
  logits -> SBUF outbuf              (PE + DVE)
preB ([b, f, tx] bf16, 16.8MB) and PP ([b, tx, 10]) are built during
the encoder via PE transposes of the h tiles, so pre never leaves the
device.
"""

import numpy as np
import time as _time

B, TX, TY = 1024, 512, 32
NA, NS = 64, 128
VIN, VOUT = 64, 32
K1 = 10
NCORES = 8
BL = B // NCORES          # 128 batch per core
TC = 16                   # time-chunk for X streaming
NCHUNK = TX // TC
FC = 4                    # feature-chunk for ctx reduce

_CACHE = {}


def _build(unused=None):
    import concourse.bass as bass
    import concourse.bacc as bacc
    import concourse.mybir as mybir
    from concourse import tile
    from concourse.masks import make_identity

    nc = bacc.Bacc("TRN2", target_bir_lowering=False, debug=False,
                   num_devices=NCORES)
    f32 = mybir.dt.float32
    bf16 = mybir.dt.bfloat16
    fp8 = mybir.dt.float8e3
    TH = mybir.ActivationFunctionType.Tanh
    EX = mybir.ActivationFunctionType.Exp
    RL = mybir.ActivationFunctionType.Relu
    CP = mybir.ActivationFunctionType.Copy
    AD, MU = mybir.AluOpType.add, mybir.AluOpType.mult
    AX = mybir.AxisListType.X

    xt = nc.dram_tensor("xt", [BL, TX, VIN], fp8, kind="ExternalInput").ap()
    wx = {}
    wh = {}
    for d in ("f", "b"):
        wx[d] = nc.dram_tensor(f"wx{d}", [VIN + 1, 4 * NA], bf16,
                               kind="ExternalInput").ap()
        wh[d] = nc.dram_tensor(f"wh{d}", [NA, 4 * NA], bf16,
                               kind="ExternalInput").ap()
    w1pp_d = nc.dram_tensor("w1pp", [NA, 2 * K1], bf16, kind="ExternalInput").ap()
    w1a_d = nc.dram_tensor("w1a", [NS, K1], bf16, kind="ExternalInput").ap()
    b1r_d = nc.dram_tensor("b1r", [1, K1], f32, kind="ExternalInput").ap()
    w2r_d = nc.dram_tensor("w2r", [1, K1], bf16, kind="ExternalInput").ap()
    wihp_d = nc.dram_tensor("wihp", [NS, 4 * NS], bf16, kind="ExternalInput").ap()
    whhp_d = nc.dram_tensor("whhp", [NS, 4 * NS], bf16, kind="ExternalInput").ap()
    bpr_d = nc.dram_tensor("bpr", [1, 4 * NS], f32, kind="ExternalInput").ap()
    w3t_d = nc.dram_tensor("w3t", [NS, VOUT], bf16, kind="ExternalInput").ap()
    b3r_d = nc.dram_tensor("b3r", [1, VOUT], f32, kind="ExternalInput").ap()
    b2s_d = nc.dram_tensor("b2s", [1, 1], f32, kind="ExternalInput").ap()

    logit = nc.dram_tensor("logit", [BL, TY * VOUT], bf16,
                           kind="ExternalOutput").ap()

    with tile.TileContext(nc) as tc:
        with (
            tc.tile_pool(name="const", bufs=1) as cpool,
            tc.tile_pool(name="big", bufs=1) as bigpool,
        ):
            # ---- persistent SBUF tensors ----
            preB = bigpool.tile([BL, NS, TX], bf16, name="preB", tag="preB")
            PP = bigpool.tile([BL, TX, K1], bf16, name="PP", tag="PP")
            nc.gpsimd.memset(PP[:], 0.0)
            outbuf = bigpool.tile([BL, TY * VOUT], bf16, name="outbuf",
                                  tag="outbuf")
            ident = cpool.tile([128, 128], bf16, name="ident", tag="ident")
            make_identity(nc, ident)

            # ---- load weights ----
            def ld(pool, dram, shape, dt, tag):
                t = pool.tile(shape, dt, name=tag, tag=tag)
                nc.sync.dma_start(t[:], dram[:])
                return t

            wx_sb = {d: ld(cpool, wx[d], [VIN + 1, 4 * NA], bf16, f"wx{d}")
                     for d in ("f", "b")}
            wh_sb = {d: ld(cpool, wh[d], [NA, 4 * NA], bf16, f"wh{d}")
                     for d in ("f", "b")}
            w1pp = ld(cpool, w1pp_d, [NA, 2 * K1], bf16, "w1pp")
            w1a = ld(cpool, w1a_d, [NS, K1], bf16, "w1a")
            wihp = ld(cpool, wihp_d, [NS, 4 * NS], bf16, "wihp")
            whhp = ld(cpool, whhp_d, [NS, 4 * NS], bf16, "whhp")
            w3t = ld(cpool, w3t_d, [NS, VOUT], bf16, "w3t")

            # small bias rows -> broadcast to all 128 partitions on device
            ones1b = cpool.tile([1, BL], bf16, name="ones1b", tag="ones1b")
            nc.gpsimd.memset(ones1b[:], 1.0)
            ones1f = cpool.tile([1, BL], f32, name="ones1f", tag="ones1f")
            nc.gpsimd.memset(ones1f[:], 1.0)

            with tc.tile_pool(name="bbc", bufs=1, space="PSUM") as bbc:
                def bcast(dram, n, dt, tag):
                    row = cpool.tile([1, n], dt, name=tag + "_r", tag=tag + "_r")
                    nc.sync.dma_start(row[:], dram[:])
                    ps = bbc.tile([BL, n], f32, name=tag + "_p", tag="bbc_ps")
                    ones1 = ones1f if dt == f32 else ones1b
                    nc.tensor.matmul(ps[:, 0:n], ones1[:], row[:],
                                     start=True, stop=True)
                    out = cpool.tile([BL, n], dt, name=tag, tag=tag)
                    nc.vector.tensor_copy(out[:], ps[:, 0:n])
                    return out

                b1r = bcast(b1r_d, K1, f32, "b1r")
                w2r = bcast(w2r_d, K1, bf16, "w2r")
                bpr = bcast(bpr_d, 4 * NS, f32, "bpr")
                b3r = bcast(b3r_d, VOUT, f32, "b3r")
                b2s = bcast(b2s_d, 1, f32, "b2s")

            # ---------------- encoder ----------------
            with (
                tc.tile_pool(name="x8", bufs=1) as x8pool,
                tc.tile_pool(name="xc", bufs=1) as xcpool,
                tc.tile_pool(name="xrow", bufs=1) as xrpool,
                tc.tile_pool(name="work", bufs=2) as wkpool,
                tc.tile_pool(name="hout", bufs=8) as hpool,
                tc.tile_pool(name="zps", bufs=2, space="PSUM") as zpsum,
                tc.tile_pool(name="pps", bufs=2, space="PSUM") as ppsum,
                tc.tile_pool(name="tps", bufs=2, space="PSUM") as tpsum,
                tc.tile_pool(name="xps", bufs=2, space="PSUM") as xpsum,
            ):
                # fp8 landing buffers + bf16 chunk buffers (double-buffered
                # per dir)
                xb8 = {}
                xbc = {}
                for d in ("f", "b"):
                    for s in (0, 1):
                        xb8[d, s] = x8pool.tile([BL, TC, VIN], fp8,
                                                name=f"x8{d}{s}",
                                                tag=f"x8{d}{s}")
                        xbc[d, s] = xcpool.tile([BL, TC, VIN], bf16,
                                                name=f"xc{d}{s}",
                                                tag=f"xc{d}{s}")
                # rotating [v+1, b] matmul operands, ones row preset
                xrow = []
                for i in range(4):
                    t = xrpool.tile([VIN + 1, BL], bf16, name=f"xr{i}",
                                    tag=f"xr{i}")
                    nc.gpsimd.memset(t[VIN:VIN + 1, :], 1.0)
                    xrow.append(t)
                xri = 0

                h0 = cpool.tile([NA, BL], bf16, name="h0", tag="h0")
                nc.gpsimd.memset(h0[:], 0.0)
                cstate = {}
                for d in ("f", "b"):
                    cstate[d] = cpool.tile([NA, BL], f32, name=f"c{d}",
                                           tag=f"c{d}")
                    nc.gpsimd.memset(cstate[d][:], 0.0)
                hprev = {"f": h0, "b": h0}

                for c in range(NCHUNK):
                    for d in ("f", "b"):
                        if d == "f":
                            ts0 = TC * c
                        else:
                            ts0 = TX - TC * (c + 1)
                        nc.sync.dma_start(xb8[d, c % 2][:],
                                          xt[:, ts0:ts0 + TC, :])
                        nc.vector.tensor_copy(xbc[d, c % 2][:],
                                              xb8[d, c % 2][:])
                    for tl in range(TC):
                        for di, d in enumerate(("f", "b")):
                            if d == "f":
                                t_act = TC * c + tl
                                xcol = tl
                            else:
                                t_act = TX - 1 - (TC * c + tl)
                                xcol = TC - 1 - tl
                            # x_t [v, b] via PE transpose
                            xT = xpsum.tile([VIN, BL], bf16, name="xT",
                                            tag="xT")
                            nc.tensor.transpose(
                                xT[:], xbc[d, c % 2][:, xcol, :], ident[:])
                            xr = xrow[xri % 4]
                            xri += 1
                            nc.scalar.copy(xr[0:VIN, :], xT[:])
                            z = zpsum.tile([NA, 4 * BL], f32, name="z", tag="z")
                            for g in range(4):
                                cs = slice(g * BL, (g + 1) * BL)
                                ws = slice(g * NA, (g + 1) * NA)
                                nc.tensor.matmul(z[:, cs], wx_sb[d][:, ws],
                                                 xr[:], start=True, stop=False)
                                nc.tensor.matmul(z[:, cs], wh_sb[d][:, ws],
                                                 hprev[d][:], start=False,
                                                 stop=True)
                            T = wkpool.tile([NA, 4 * BL], f32, name="T", tag="T")
                            nc.scalar.activation(T[:], z[:], TH)
                            ti = T[:, 0:BL]
                            tf = T[:, BL:2 * BL]
                            tg = T[:, 2 * BL:3 * BL]
                            to = T[:, 3 * BL:4 * BL]
                            m1 = wkpool.tile([NA, BL], f32, name="m1", tag="m1")
                            m2 = wkpool.tile([NA, BL], f32, name="m2", tag="m2")
                            # C' = 2c; m1=(tf+1)*C'; m2=(ti+1)*tg;
                            # C'new = 0.5*m1 + m2
                            nc.vector.scalar_tensor_tensor(
                                m1[:], tf, 1.0, cstate[d][:], AD, MU)
                            nc.vector.scalar_tensor_tensor(
                                m2[:], ti, 1.0, tg, AD, MU)
                            nc.vector.scalar_tensor_tensor(
                                cstate[d][:], m1[:], 0.5, m2[:], MU, AD)
                            tcell = wkpool.tile([NA, BL], f32, name="tc",
                                                tag="tc")
                            nc.scalar.activation(tcell[:], cstate[d][:], TH,
                                                 scale=0.5)
                            hnew = hpool.tile([NA, BL], bf16, name="h", tag="h")
                            # H' = (to+1)*tanh(c) = 2h
                            nc.vector.scalar_tensor_tensor(
                                hnew[:], to, 1.0, tcell[:], AD, MU)
                            hprev[d] = hnew
                            # PP[:, t, :] += hT @ w1pp_d  (PE, N=10)
                            pp_ps = ppsum.tile([BL, K1], f32, name="pp",
                                               tag="pp")
                            nc.tensor.matmul(
                                pp_ps[:], hnew[:],
                                w1pp[:, di * K1:(di + 1) * K1],
                                start=True, stop=True)
                            nc.vector.tensor_tensor(
                                PP[:, t_act, :], pp_ps[:], PP[:, t_act, :], AD)
                            # preB[:, d*64:(d+1)*64, t] = hnew.T
                            hT = tpsum.tile([BL, NA], bf16, name="hT", tag="hT")
                            nc.tensor.transpose(hT[:], hnew[:],
                                                ident[0:NA, 0:NA])
                            nc.scalar.copy(
                                preB[:, di * NA:(di + 1) * NA, t_act], hT[:])

            # ---------------- decoder ----------------
            with (
                tc.tile_pool(name="dwork", bufs=1) as dpool,
                tc.tile_pool(name="ebuf", bufs=1) as epool,
                tc.tile_pool(name="prod", bufs=2) as prpool,
                tc.tile_pool(name="dz", bufs=2, space="PSUM") as dzps,
                tc.tile_pool(name="dsm", bufs=1, space="PSUM") as dsps,
                tc.tile_pool(name="dtr", bufs=1, space="PSUM") as dtps,
            ):
                s_fT = cpool.tile([NS, BL], bf16, name="sfT", tag="sfT")
                nc.gpsimd.memset(s_fT[:], 0.0)
                cdec = cpool.tile([BL, NS], f32, name="cdec", tag="cdec")
                nc.gpsimd.memset(cdec[:], 0.0)
                e = epool.tile([BL, TX, K1], bf16, name="e", tag="e")

                for t in range(TY):
                    # PS = s'@w1a + b1
                    ps_ps = dsps.tile([BL, K1], f32, name="ps", tag="ps")
                    nc.tensor.matmul(ps_ps[:], s_fT[:], w1a[:],
                                     start=True, stop=True)
                    PS = dpool.tile([BL, K1], f32, name="PS", tag="PS")
                    nc.vector.tensor_tensor(PS[:], ps_ps[:], b1r[:], AD)
                    # e = tanh(PP + PS)
                    nc.vector.tensor_tensor(
                        e[:], PP[:],
                        PS[:, None, :].broadcast_to([BL, TX, K1]), AD)
                    nc.scalar.activation(e[:], e[:], TH)
                    # q = sum_k e*W2 ; u = relu(q + b2); w = exp(u), Z
                    nc.vector.tensor_tensor(
                        e[:], e[:],
                        w2r[:, None, :].broadcast_to([BL, TX, K1]), MU)
                    q = dpool.tile([BL, TX], f32, name="q", tag="q")
                    nc.vector.tensor_reduce(q[:], e[:], AX, AD)
                    u = dpool.tile([BL, TX], f32, name="u", tag="u")
                    nc.scalar.activation(u[:], q[:], RL, bias=b2s)
                    wat = dpool.tile([BL, TX], bf16, name="wat", tag="wat")
                    Z = dpool.tile([BL, 1], f32, name="Z", tag="Z")
                    nc.scalar.activation(wat[:], u[:], EX, accum_out=Z[:])
                    # ctx_u[f] = sum_tx w*preB
                    ctx_u = dpool.tile([BL, NS], f32, name="ctxu", tag="ctxu")
                    for fc in range(NS // FC):
                        fs = slice(fc * FC, (fc + 1) * FC)
                        prod = prpool.tile([BL, FC, TX], bf16, name="prod",
                                           tag="prod")
                        nc.vector.tensor_tensor(
                            prod[:], preB[:, fs, :],
                            wat[:, None, :].broadcast_to([BL, FC, TX]), MU)
                        nc.vector.tensor_reduce(ctx_u[:, fs], prod[:], AX, AD)
                    # ctx = ctx_u / Z  (ACT copy w/ per-partition scale)
                    Zr = dpool.tile([BL, 1], f32, name="Zr", tag="Zr")
                    nc.vector.reciprocal(Zr[:], Z[:])
                    ctx = dpool.tile([BL, NS], bf16, name="ctx", tag="ctx")
                    nc.scalar.activation(ctx[:], ctx_u[:], CP, scale=Zr)
                    # ctx_fT
                    ctT_ps = dtps.tile([NS, BL], bf16, name="ctT", tag="ctT")
                    nc.tensor.transpose(ctT_ps[:], ctx[:], ident[:])
                    ctx_fT = dpool.tile([NS, BL], bf16, name="cfT", tag="cfT")
                    nc.vector.tensor_copy(ctx_fT[:], ctT_ps[:])
                    # z = ctx@wihp + s@whhp + bp
                    z_ps = dzps.tile([BL, 4 * NS], f32, name="zd", tag="zd")
                    nc.tensor.matmul(z_ps[:], ctx_fT[:], wihp[:],
                                     start=True, stop=False)
                    nc.tensor.matmul(z_ps[:], s_fT[:], whhp[:],
                                     start=False, stop=True)
                    zb = dpool.tile([BL, 4 * NS], f32, name="zb", tag="zb")
                    nc.vector.tensor_tensor(zb[:], z_ps[:], bpr[:], AD)
                    # gates: i,f scale 0.5 tanh; g scale 1; o scale 0.5
                    Tg8 = dpool.tile([BL, 4 * NS], f32, name="Tg8", tag="Tg8")
                    nc.scalar.activation(Tg8[:, 0:2 * NS], zb[:, 0:2 * NS],
                                         TH, scale=0.5)
                    nc.scalar.activation(Tg8[:, 2 * NS:3 * NS],
                                         zb[:, 2 * NS:3 * NS], TH)
                    nc.scalar.activation(Tg8[:, 3 * NS:4 * NS],
                                         zb[:, 3 * NS:4 * NS], TH, scale=0.5)
                    ti = Tg8[:, 0:NS]
                    tf = Tg8[:, NS:2 * NS]
                    tg = Tg8[:, 2 * NS:3 * NS]
                    to = Tg8[:, 3 * NS:4 * NS]
                    m1 = dpool.tile([BL, NS], f32, name="dm1", tag="dm1")
                    m2 = dpool.tile([BL, NS], f32, name="dm2", tag="dm2")
                    nc.vector.scalar_tensor_tensor(m1[:], tf, 1.0, cdec[:],
                                                   AD, MU)
                    nc.vector.scalar_tensor_tensor(m2[:], ti, 1.0, tg, AD, MU)
                    nc.vector.scalar_tensor_tensor(cdec[:], m1[:], 0.5, m2[:],
                                                   MU, AD)
                    tcl = dpool.tile([BL, NS], f32, name="dtc", tag="dtc")
                    nc.scalar.activation(tcl[:], cdec[:], TH, scale=0.5)
                    s_b = dpool.tile([BL, NS], bf16, name="sb", tag="sb")
                    nc.vector.scalar_tensor_tensor(s_b[:], to, 1.0, tcl[:],
                                                   AD, MU)
                    # s_fT = s_b.T
                    sT_ps = dtps.tile([NS, BL], bf16, name="sT", tag="sT")
                    nc.tensor.transpose(sT_ps[:], s_b[:], ident[:])
                    nc.vector.tensor_copy(s_fT[:], sT_ps[:])
                    # logits
                    L_ps = dsps.tile([BL, VOUT], f32, name="L", tag="L")
                    nc.tensor.matmul(L_ps[:], s_fT[:], w3t[:],
                                     start=True, stop=True)
                    nc.vector.tensor_tensor(
                        outbuf[:, t * VOUT:(t + 1) * VOUT], L_ps[:], b3r[:],
                        AD)

                nc.sync.dma_start(logit[:], outbuf[:])

    nc.compile()
    return nc


def _get_nc():
    if "nc" not in _CACHE:
        _CACHE["nc"] = _build()
    return _CACHE["nc"]


def _prep_enc_weights(Wih, Whh, bih, bhh, bf16):
    """Baseline folding: sigmoid-from-tanh 0.5 scales + bias row; Whh gets
    an extra 0.5 because the h it multiplies is stored doubled."""
    b = (bih + bhh).astype(np.float32)
    scale = np.concatenate([np.full(2 * NA, 0.5, np.float32),
                            np.full(NA, 1.0, np.float32),
                            np.full(NA, 0.5, np.float32)])
    Wx = (Wih * scale[:, None]).astype(np.float32)
    Wh = (Whh * (0.5 * scale)[:, None]).astype(np.float32)
    bb = (b * scale).astype(np.float32)
    wx_aug = np.concatenate([Wx.T, bb[None, :]], axis=0)
    return (np.ascontiguousarray(wx_aug).astype(bf16),
            np.ascontiguousarray(Wh.T).astype(bf16))


def _run_cached(nc, in_maps):
    import jax
    import numpy as _np
    from jax.sharding import Mesh, PartitionSpec
    from jax.experimental.shard_map import shard_map
    from concourse import bass2jax as b2j

    if "runner" not in _CACHE:
        b2j.install_neuronx_cc_hook()
        import concourse.mybir as mybir
        pname = (nc.partition_id_tensor.name
                 if nc.partition_id_tensor else None)
        in_names, out_names, out_avals = [], [], []
        for alloc in nc.m.functions[0].allocations:
            if not isinstance(alloc, mybir.MemoryLocationSet):
                continue
            name = alloc.memorylocations[0].name
            if alloc.kind == "ExternalInput":
                if name != pname:
                    in_names.append(name)
            elif alloc.kind == "ExternalOutput":
                out_names.append(name)
                out_avals.append(jax.core.ShapedArray(
                    tuple(alloc.tensor_shape), mybir.dt.np(alloc.dtype)))
        n_params = len(in_names)
        all_names = in_names + out_names
        if pname is not None:
            all_names = all_names + [pname]

        def _body(*args):
            ops = list(args)
            if pname is not None:
                ops.append(b2j.partition_id_tensor())
            outs = b2j._bass_exec_p.bind(
                *ops, out_avals=tuple(out_avals), in_names=tuple(all_names),
                out_names=tuple(out_names), lowering_input_output_aliases=(),
                sim_require_finite=True, sim_require_nnan=True, nc=nc)
            return tuple(outs)

        devices = jax.devices()[:NCORES]
        mesh = Mesh(_np.asarray(devices), ("core",))
        nio = n_params + len(out_names)
        sharded = jax.jit(
            shard_map(_body, mesh=mesh,
                      in_specs=(PartitionSpec("core"),) * nio,
                      out_specs=(PartitionSpec("core"),) * len(out_names),
                      check_rep=False),
            donate_argnums=tuple(range(n_params, nio)), keep_unused=True)
        _CACHE["runner"] = (sharded, in_names, out_names, out_avals, n_params)

    sharded, in_names, out_names, out_avals, n_params = _CACHE["runner"]
    concat_in = [_np.concatenate([_np.asarray(m[n]) for m in in_maps], axis=0)
                 for n in in_names]
    concat_zeros = [
        _np.zeros((NCORES * a.shape[0], *a.shape[1:]), a.dtype)
        for a in out_avals]
    out_arrs = sharded(*concat_in, *concat_zeros)
    return [
        {n: _np.asarray(out_arrs[i]).reshape(NCORES, *out_avals[i].shape)[c]
         for i, n in enumerate(out_names)}
        for c in range(NCORES)
    ]


def kernel(X, Wih_f, Whh_f, bih_f, bhh_f, Wih_b, Whh_b, bih_b, bhh_b,
           Wih_p, Whh_p, bih_p, bhh_p, W1, b1, W2, b2, W3, b3):
    import ml_dtypes
    bf16 = ml_dtypes.bfloat16
    fp8 = ml_dtypes.float8_e3m4

    _t = {}
    _t0 = _time.time()
    nc = _get_nc()
    _t['build'] = _time.time() - _t0
    _t0 = _time.time()

    wf = _prep_enc_weights(Wih_f, Whh_f, bih_f, bhh_f, bf16)
    wb = _prep_enc_weights(Wih_b, Whh_b, bih_b, bhh_b, bf16)

    W1 = np.asarray(W1, np.float32)
    # decoder weights (shared across cores). All "0.5" folds: the doubled
    # h/s/pre states.
    w1pp = np.concatenate([0.5 * W1[:, NS:NS + NA].T,
                           0.5 * W1[:, NS + NA:2 * NS].T],
                          axis=1).astype(bf16)                   # (64, 20)
    w1a = np.ascontiguousarray(0.5 * W1[:, :NS].T).astype(bf16)  # (128, 10)
    b1r = np.asarray(b1, np.float32).reshape(1, K1)
    w2r = np.asarray(W2[0], np.float32).astype(bf16).reshape(1, K1)
    wihp = np.ascontiguousarray(0.5 * np.asarray(Wih_p, np.float32).T
                                ).astype(bf16)                   # (128, 512)
    whhp = np.ascontiguousarray(0.5 * np.asarray(Whh_p, np.float32).T
                                ).astype(bf16)
    bpr = (np.asarray(bih_p, np.float32)
           + np.asarray(bhh_p, np.float32)).reshape(1, 4 * NS)
    w3t = np.ascontiguousarray(0.5 * np.asarray(W3, np.float32).T
                               ).astype(bf16)                    # (128, 32)
    b3r = np.asarray(b3, np.float32).reshape(1, VOUT)
    b2s = np.full((1, 1), float(np.asarray(b2).ravel()[0]), np.float32)

    X8 = np.asarray(X).astype(fp8)                   # (1024, 512, 64) fp8
    in_maps = []
    for c in range(NCORES):
        m = {"xt": X8[c * BL:(c + 1) * BL], "w1pp": w1pp, "w1a": w1a,
             "b1r": b1r, "w2r": w2r, "wihp": wihp, "whhp": whhp, "bpr": bpr,
             "w3t": w3t, "b3r": b3r, "b2s": b2s}
        for d, w in (("f", wf), ("b", wb)):
            m[f"wx{d}"] = w[0]
            m[f"wh{d}"] = w[1]
        in_maps.append(m)

    _t['prep'] = _time.time() - _t0
    _t0 = _time.time()
    try:
        results = _run_cached(nc, in_maps)
    except Exception:
        from concourse.bass_utils import run_bass_kernel_spmd
        results = run_bass_kernel_spmd(
            nc, in_maps, core_ids=list(range(NCORES))).results
    _t['spmd'] = _time.time() - _t0
    _t0 = _time.time()
    _CACHE["last_results"] = results
    _CACHE["last_in_maps"] = in_maps

    # logits (B, TY, VOUT); final softmax over batch axis
    L = np.concatenate(
        [results[c]["logit"].reshape(BL, TY, VOUT).astype(np.float32)
         for c in range(NCORES)], axis=0)
    L -= L.max(axis=0, keepdims=True)
    np.exp(L, out=L)
    L /= L.sum(axis=0, keepdims=True)
    _t['post'] = _time.time() - _t0
    _CACHE['timers'] = _t
    return np.ascontiguousarray(L)
